# revision 21
# baseline (speedup 1.0000x reference)
"""Causal self-attention Trainium2 Bass kernel (v3, bf16).

Problem (hardcoded): B=4, S=2048, D=1024, H=16 heads, head_dim=64.
    qkv = x @ W_attn + b_attn; causal softmax attention; y @ W_proj + b_proj.

Sharding over 8 NeuronCores: core c -> (batch b = c//2, head-group g = c%2).
Each core computes, for its batch and its 8 heads (512 feature dims):
    Q^T, K^T [512f, 2048s] and V [2048s, 512f] in bf16
    flash-style causal attention in transposed layout, per head:
        scores^T [128k, 512q] = K^T.T @ Q^T  (two heads concurrent via PE
        row groups 0/64), exp on ACT (bf16 out), causal mask for diagonal
        blocks via gpsimd affine_select, PV accumulation [65hd, 512q] with a
        ones column carrying the softmax denominator.
    normalization: DVE reciprocal of the denominator row, gpsimd
    partition_broadcast, DVE multiply writing bf16 y^T.
    projection: y^T.T @ W_proj -> [2048, 1024] fp32 partial.
All matmuls bf16 (separate LDWEIGHTS with FWL overlaps the previous matmul;
fp32r would self-load weights at ~180ns serialized per matmul).
QKV and projection matmuls are interleaved into the attention stream at
sub-tile granularity so the PE never idles while ACT exp catches up.
Host: out[b] = partial(core 2b) + partial(core 2b+1) + b_proj + b_attn_v @ W_proj.
"""
import sys
if '/opt/trn_rl_repo' not in sys.path:
    sys.path.insert(0, '/opt/trn_rl_repo')

import numpy as np
import ml_dtypes
import concourse.bass as bass
import concourse.mybir as mybir
import concourse.tile as tile
from concourse import bacc
from concourse import bass_utils
from concourse import library_config

F32 = mybir.dt.float32
BF16 = mybir.dt.bfloat16
AF = mybir.ActivationFunctionType
ALU = mybir.AluOpType

B, S, D, H, HD = 4, 2048, 1024, 16, 64
NCORES = 8
FPC = 512            # feature dims per core (8 heads * 64)
NPAIR = 4            # head pairs per core
DC = D // 128        # 8 contraction chunks
NST = S // 128       # 16 s-tiles

_CACHE = {}


def _build_program():
    nc = bacc.Bacc("TRN2", target_bir_lowering=False, debug=False,
                   enable_asserts=False, num_devices=NCORES)

    xT_d = nc.dram_tensor("xT", [D, S], BF16, kind="ExternalInput").ap()
    wq_d = nc.dram_tensor("wq", [D, FPC], BF16, kind="ExternalInput").ap()
    wk_d = nc.dram_tensor("wk", [D, FPC], BF16, kind="ExternalInput").ap()
    wv_d = nc.dram_tensor("wv", [D, FPC], BF16, kind="ExternalInput").ap()
    wp_d = nc.dram_tensor("wp", [FPC, D], BF16, kind="ExternalInput").ap()
    bq_d = nc.dram_tensor("bq", [FPC], F32, kind="ExternalInput").ap()
    bk_d = nc.dram_tensor("bk", [FPC], F32, kind="ExternalInput").ap()
    out_d = nc.dram_tensor("out", [S, D], F32, kind="ExternalOutput").ap()

    from contextlib import ExitStack
    with tile.TileContext(nc) as tc, ExitStack() as ctx:
        persist = ctx.enter_context(tc.tile_pool(name="persist", bufs=1))
        xpool = ctx.enter_context(tc.tile_pool(name="xpool", bufs=2))
        expool = ctx.enter_context(tc.tile_pool(name="expool", bufs=4))
        smpool = ctx.enter_context(tc.tile_pool(name="smpool", bufs=3))
        outsb = ctx.enter_context(tc.tile_pool(name="outsb", bufs=3))
        scps = ctx.enter_context(tc.tile_pool(name="scps", bufs=2, space="PSUM"))
        wps = ctx.enter_context(tc.tile_pool(name="wps", bufs=2, space="PSUM"))
        accps = ctx.enter_context(tc.tile_pool(name="accps", bufs=2, space="PSUM"))

        nc.gpsimd.load_library(library_config.attn)

        QT = [persist.tile([128, S], BF16, name=f"qt{p}") for p in range(NPAIR)]
        KT = [persist.tile([128, S], BF16, name=f"kt{p}") for p in range(NPAIR)]
        yT = [persist.tile([128, S], BF16, name=f"yt{p}") for p in range(NPAIR)]
        # V tiles: [128 s, 8 heads, 65] -- col 64 is the ones column (denominator)
        Vt = [persist.tile([128, 8, 65], BF16, name=f"v{i}") for i in range(NST)]

        # Inputs needed first (x chunk 0, W_q) are issued first in halves so
        # the first matmuls aren't stuck behind the full 5MB of input DMA
        # competing for HBM bandwidth.
        xq0 = xpool.tile([128, DC, 512], BF16, name="xq_seg0", tag="xq")
        wq_sb = persist.tile([128, DC, FPC], BF16, name="wq_sb")
        wk_sb = persist.tile([128, DC, FPC], BF16, name="wk_sb")
        wv_sb = persist.tile([128, DC, FPC], BF16, name="wv_sb")
        wp_sb = persist.tile([128, 4, D], BF16, name="wp_sb")
        hc = DC // 2
        for h in range(2):
            cs = slice(512 * h, 512 * h + 512)
            nc.sync.dma_start(
                xq0[:, hc * h:hc * h + hc, :],
                xT_d[cs, 0:512].rearrange("(c p) s -> p c s", p=128))
            nc.sync.dma_start(
                wq_sb[:, hc * h:hc * h + hc, :],
                wq_d[cs, :].rearrange("(c p) f -> p c f", p=128))
        nc.sync.dma_start(wk_sb[:], wk_d.rearrange("(c p) f -> p c f", p=128))
        nc.sync.dma_start(wv_sb[:], wv_d.rearrange("(c p) f -> p c f", p=128))
        bq_sb = persist.tile([128, 4], F32, name="bq_sb")
        bk_sb = persist.tile([128, 4], F32, name="bk_sb")
        nc.sync.dma_start(bq_sb[:], bq_d.rearrange("(c p) -> p c", p=128))
        nc.sync.dma_start(bk_sb[:], bk_d.rearrange("(c p) -> p c", p=128))
        nc.sync.dma_start(wp_sb[:], wp_d.rearrange("(c p) f -> p c f", p=128))

        onesv = persist.tile([128, 8], BF16, name="onesv")
        nc.gpsimd.memset(onesv[:], 1.0)
        for i in range(NST):
            nc.vector.tensor_copy(Vt[i][:, :, 64], onesv[:])

        # ---- emission helpers ------------------------------------------
        def qk_units(seg, p, xq):
            """4 closures: Q(p) first/second half, K(p) first/second half."""
            s0 = 512 * seg
            st = {}

            def mk(nm, w_sb, b_sb, dstT):
                def u0():
                    ps = wps.tile([128, 512], F32, tag="wps",
                                  name=f"ps{nm}{seg}_{p}")
                    for c in range(4):
                        nc.tensor.matmul(ps[:], w_sb[:, c, 128 * p:128 * p + 128],
                                         xq[:, c, :], start=(c == 0), stop=False)
                    st[nm] = ps

                def u1():
                    ps = st[nm]
                    for c in range(4, DC):
                        nc.tensor.matmul(ps[:], w_sb[:, c, 128 * p:128 * p + 128],
                                         xq[:, c, :], start=False,
                                         stop=(c == DC - 1))
                    nc.vector.tensor_scalar_add(dstT[p][:, s0:s0 + 512], ps[:],
                                                b_sb[:, p:p + 1])
                return [u0, u1]

            return mk("q", wq_sb, bq_sb, QT) + mk("k", wk_sb, bk_sb, KT)

        def v_units(seg, xq):
            """4 closures, one V s-tile each."""
            us = []
            for ii in range(4):
                i = 4 * seg + ii

                def u(i=i, ii=ii):
                    ps = wps.tile([128, 512], F32, tag="wps", name=f"psv{i}")
                    for c in range(DC):
                        nc.tensor.matmul(ps[:], xq[:, c, 128 * ii:128 * ii + 128],
                                         wv_sb[:, c, :], start=(c == 0),
                                         stop=(c == DC - 1))
                    nc.vector.tensor_copy(
                        Vt[i][:, :, 0:64],
                        ps[:].rearrange("p (h u) -> p h u", h=8))
                us.append(u)
            return us

        def proj_units(j):
            """8 closures, one [128s, 512d] output tile each."""
            us = []
            for i4 in range(4):
                for o in range(2):
                    i = 4 * j + i4

                    def u(i=i, o=o):
                        po = wps.tile([128, 512], F32, tag="wps",
                                      name=f"po{i}_{o}")
                        for p2 in range(NPAIR):
                            nc.tensor.matmul(po[:],
                                             yT[p2][:, 128 * i:128 * i + 128],
                                             wp_sb[:, p2, 512 * o:512 * o + 512],
                                             start=(p2 == 0), stop=(p2 == 3))
                        ot = outsb.tile([128, 512], F32, tag="ot",
                                        name=f"ot{i}_{o}")
                        nc.vector.tensor_copy(ot[:], po[:])
                        nc.sync.dma_start(
                            out_d[128 * i:128 * i + 128, 512 * o:512 * o + 512],
                            ot[:])
                    us.append(u)
            return us

        def att_pair(j, p, inject):
            q0 = 512 * j
            nk = 4 * (j + 1)
            accA = accps.tile([65, 512], F32, tag="acc", name=f"accA{j}_{p}")
            accB = accps.tile([65, 512], F32, tag="acc", name=f"accB{j}_{p}")

            def emit_pv(t, ex, lo):
                nc.tensor.matmul(accA[:, lo:512], Vt[t][:, 2 * p, :],
                                 ex[:, lo:512], start=(t == 0),
                                 stop=(t == nk - 1))
                nc.tensor.matmul(accB[:, lo:512], Vt[t][:, 2 * p + 1, :],
                                 ex[:, 512 + lo:1024], start=(t == 0),
                                 stop=(t == nk - 1))

            pending = None
            for t in range(nk):
                k0 = 128 * t
                oi = t - 4 * j
                lo = max(0, 128 * oi)
                sc = scps.tile([128, 1024], F32, tag="sc", name=f"sc{j}_{p}_{t}")
                nc.tensor.matmul(sc[:, lo:512], KT[p][0:64, k0:k0 + 128],
                                 QT[p][0:64, q0 + lo:q0 + 512],
                                 start=True, stop=True)
                nc.tensor.matmul(sc[:, 512 + lo:1024], KT[p][64:128, k0:k0 + 128],
                                 QT[p][64:128, q0 + lo:q0 + 512],
                                 start=True, stop=True)
                ex = expool.tile([128, 1024], BF16, tag="ex",
                                 name=f"ex{j}_{p}_{t}")
                if oi < 0:
                    nc.scalar.activation(ex[:], sc[:], AF.Exp, scale=0.125)
                else:
                    if lo <= 256:
                        # one ACT op; the dead zone costs less than a 2nd
                        # op's fixed overhead at these widths
                        nc.scalar.activation(ex[:, lo:1024], sc[:, lo:1024],
                                             AF.Exp, scale=0.125)
                    else:
                        nc.scalar.activation(ex[:, lo:512], sc[:, lo:512],
                                             AF.Exp, scale=0.125)
                        nc.scalar.activation(ex[:, 512 + lo:1024],
                                             sc[:, 512 + lo:1024], AF.Exp,
                                             scale=0.125)
                    # strict upper triangle of the diagonal block
                    for lo2 in (lo, 512 + lo):
                        nc.gpsimd.affine_select(
                            out=ex[:, lo2:lo2 + 128], in_=ex[:, lo2:lo2 + 128],
                            compare_op=ALU.is_ge, fill=0.0,
                            base=0, pattern=[[1, 128]], channel_multiplier=-1)
                if pending is not None:
                    emit_pv(*pending)
                pending = (t, ex, lo)
                inject()
            emit_pv(*pending)

            # normalization: evacuate PSUM fast (recip + unnormalized copy),
            # then broadcast the reciprocal and scale yT in place — the PE
            # and the acc slots never wait on the broadcast
            denA = smpool.tile([1, 512], F32, tag="denA", name=f"denA{j}_{p}")
            denB = smpool.tile([1, 512], F32, tag="denB", name=f"denB{j}_{p}")
            recA = smpool.tile([1, 512], F32, tag="rec", name=f"recA{j}_{p}")
            recB = smpool.tile([1, 512], F32, tag="rec", name=f"recB{j}_{p}")
            bcA = smpool.tile([128, 512], F32, tag="bcA", name=f"bcA{j}_{p}")
            bcB = smpool.tile([128, 512], F32, tag="bcB", name=f"bcB{j}_{p}")
            for acc, den, rec, bc, hi in ((accA, denA, recA, bcA, 0),
                                          (accB, denB, recB, bcB, 1)):
                ys = yT[p][64 * hi:64 * hi + 64, q0:q0 + 512]
                nc.vector.tensor_copy(den[:], acc[64:65, :])
                nc.vector.reciprocal_approx_fast(rec[:], den[:])
                nc.vector.tensor_copy(ys, acc[0:64, :])
                nc.gpsimd.partition_broadcast(bc[:], rec[0:1, :], channels=128)
            for bc, hi in ((bcA, 0), (bcB, 1)):
                ys = yT[p][64 * hi:64 * hi + 64, q0:q0 + 512]
                nc.gpsimd.tensor_tensor(ys, ys,
                                        bc[64 * hi:64 * hi + 64, :], ALU.mult)

        # ---- main schedule ---------------------------------------------
        # Segment seg's attention stream absorbs, at sub-tile granularity:
        # this segment's remaining QK projections, the NEXT segment's full
        # QKV (so no PE-only stretches remain between segments), and the
        # PREVIOUS chunk's output projection.
        xqs = [xq0]
        for seg in range(1, 4):
            xqs.append(xpool.tile([128, DC, 512], BF16, name=f"xq{seg}",
                                  tag="xq"))

        for seg in range(4):
            if seg < 3:
                s1 = 512 * (seg + 1)
                nc.sync.dma_start(
                    xqs[seg + 1][:],
                    xT_d[:, s1:s1 + 512].rearrange("(c p) s -> p c s", p=128))
            xq = xqs[seg]
            if seg == 0:   # nothing earlier to hide these under
                for u in qk_units(0, 0, xq):
                    u()
                for u in v_units(0, xq):
                    u()
            queues = [[] for _ in range(NPAIR)]
            for pp in (1, 2, 3):
                queues[pp - 1] += qk_units(seg, pp, xq)
            pu = proj_units(seg - 1) if seg >= 1 else []
            nxt = []
            if seg < 3:
                nxt += v_units(seg + 1, xqs[seg + 1])
                nxt += qk_units(seg + 1, 0, xqs[seg + 1])
            queues[1] += pu[0:4] + nxt[0:2]
            queues[2] += pu[4:8] + nxt[2:5]
            queues[3] += nxt[5:8]
            for p in range(NPAIR):
                q = queues[p]

                def inject(q=q):
                    if q:
                        q.pop(0)()
                att_pair(seg, p, inject)
                while q:   # flush any leftovers at pair end
                    q.pop(0)()
        for u in proj_units(3):
            u()

    nc.compile()
    return nc


def _get_program():
    if "nc" not in _CACHE:
        _CACHE["nc"] = _build_program()
    return _CACHE["nc"]


def kernel(x, W_attn, b_attn, W_proj, b_proj, _trace=False, _trace_cores=None):
    x = np.asarray(x, np.float32)
    W_attn = np.asarray(W_attn, np.float32)
    b_attn = np.asarray(b_attn, np.float32)
    W_proj = np.asarray(W_proj, np.float32)
    b_proj = np.asarray(b_proj, np.float32)

    nc = _get_program()

    bf16 = ml_dtypes.bfloat16
    x16 = x.astype(bf16)
    Wa16 = W_attn.astype(bf16)
    Wp16 = W_proj.astype(bf16)

    in_maps = []
    for c in range(NCORES):
        b, g = divmod(c, 2)
        gc = slice(FPC * g, FPC * g + FPC)
        in_maps.append({
            "xT": np.ascontiguousarray(x16[b].T),
            "wq": np.ascontiguousarray(Wa16[:, 0 * D:1 * D][:, gc]),
            "wk": np.ascontiguousarray(Wa16[:, 1 * D:2 * D][:, gc]),
            "wv": np.ascontiguousarray(Wa16[:, 2 * D:3 * D][:, gc]),
            "wp": np.ascontiguousarray(Wp16[gc, :]),
            "bq": np.ascontiguousarray(b_attn[0 * D:1 * D][gc]),
            "bk": np.ascontiguousarray(b_attn[1 * D:2 * D][gc]),
        })

    kw = {}
    if _trace:
        kw = dict(trace=True, trace_cores=_trace_cores or [0])
    res = bass_utils.run_bass_kernel_spmd(nc, in_maps, core_ids=list(range(NCORES)),
                                          **kw)

    # host-side reduction: v-bias commutes through softmax -> fold via W_proj
    corr = b_proj + b_attn[2 * D:3 * D] @ W_proj
    out = np.empty((B, S, D), np.float32)
    for b in range(B):
        out[b] = res.results[2 * b]["out"] + res.results[2 * b + 1]["out"] + corr

    if _trace:
        kernel._last_results = res
    return out


# revision 22
# speedup vs baseline: 2.3625x; 2.3625x over previous
"""Causal self-attention Trainium2 Bass kernel (v3, bf16).

Problem (hardcoded): B=4, S=2048, D=1024, H=16 heads, head_dim=64.
    qkv = x @ W_attn + b_attn; causal softmax attention; y @ W_proj + b_proj.

Sharding over 8 NeuronCores: core c -> (batch b = c//2, head-group g = c%2).
Each core computes, for its batch and its 8 heads (512 feature dims):
    Q^T, K^T [512f, 2048s] and V [2048s, 512f] in bf16
    flash-style causal attention in transposed layout, per head:
        scores^T [128k, 512q] = K^T.T @ Q^T  (two heads concurrent via PE
        row groups 0/64), exp on ACT (bf16 out), causal mask for diagonal
        blocks via gpsimd affine_select, PV accumulation [65hd, 512q] with a
        ones column carrying the softmax denominator.
    normalization: DVE reciprocal of the denominator row, gpsimd
    partition_broadcast, DVE multiply writing bf16 y^T.
    projection: y^T.T @ W_proj -> [2048, 1024] fp32 partial.
All matmuls bf16 (separate LDWEIGHTS with FWL overlaps the previous matmul;
fp32r would self-load weights at ~180ns serialized per matmul).
QKV and projection matmuls are interleaved into the attention stream at
sub-tile granularity so the PE never idles while ACT exp catches up.
Host: out[b] = partial(core 2b) + partial(core 2b+1) + b_proj + b_attn_v @ W_proj.
"""
import sys
if '/opt/trn_rl_repo' not in sys.path:
    sys.path.insert(0, '/opt/trn_rl_repo')

import numpy as np
import ml_dtypes
import concourse.bass as bass
import concourse.mybir as mybir
import concourse.tile as tile
from concourse import bacc
from concourse import bass_utils
from concourse import library_config

F32 = mybir.dt.float32
BF16 = mybir.dt.bfloat16
AF = mybir.ActivationFunctionType
ALU = mybir.AluOpType

B, S, D, H, HD = 4, 2048, 1024, 16, 64
NCORES = 8
FPC = 512            # feature dims per core (8 heads * 64)
NPAIR = 4            # head pairs per core
DC = D // 128        # 8 contraction chunks
NST = S // 128       # 16 s-tiles

_CACHE = {}


def _build_program():
    nc = bacc.Bacc("TRN2", target_bir_lowering=False, debug=False,
                   enable_asserts=False, num_devices=NCORES)

    xT_d = nc.dram_tensor("xT", [D, S], BF16, kind="ExternalInput").ap()
    wq_d = nc.dram_tensor("wq", [D, FPC], BF16, kind="ExternalInput").ap()
    wk_d = nc.dram_tensor("wk", [D, FPC], BF16, kind="ExternalInput").ap()
    wv_d = nc.dram_tensor("wv", [D, FPC], BF16, kind="ExternalInput").ap()
    wp_d = nc.dram_tensor("wp", [FPC, D], BF16, kind="ExternalInput").ap()
    bq_d = nc.dram_tensor("bq", [FPC], F32, kind="ExternalInput").ap()
    bk_d = nc.dram_tensor("bk", [FPC], F32, kind="ExternalInput").ap()
    out_d = nc.dram_tensor("out", [S, D], F32, kind="ExternalOutput").ap()

    from contextlib import ExitStack
    with tile.TileContext(nc) as tc, ExitStack() as ctx:
        persist = ctx.enter_context(tc.tile_pool(name="persist", bufs=1))
        xpool = ctx.enter_context(tc.tile_pool(name="xpool", bufs=2))
        expool = ctx.enter_context(tc.tile_pool(name="expool", bufs=4))
        smpool = ctx.enter_context(tc.tile_pool(name="smpool", bufs=3))
        outsb = ctx.enter_context(tc.tile_pool(name="outsb", bufs=3))
        scps = ctx.enter_context(tc.tile_pool(name="scps", bufs=2, space="PSUM"))
        wps = ctx.enter_context(tc.tile_pool(name="wps", bufs=2, space="PSUM"))
        accps = ctx.enter_context(tc.tile_pool(name="accps", bufs=2, space="PSUM"))

        nc.gpsimd.load_library(library_config.attn)

        QT = [persist.tile([128, S], BF16, name=f"qt{p}") for p in range(NPAIR)]
        KT = [persist.tile([128, S], BF16, name=f"kt{p}") for p in range(NPAIR)]
        yT = [persist.tile([128, S], BF16, name=f"yt{p}") for p in range(NPAIR)]
        # V tiles: [128 s, 8 heads, 65] -- col 64 is the ones column (denominator)
        Vt = [persist.tile([128, 8, 65], BF16, name=f"v{i}") for i in range(NST)]

        # Inputs needed first (x chunk 0, W_q) are issued first in halves so
        # the first matmuls aren't stuck behind the full 5MB of input DMA
        # competing for HBM bandwidth.
        xq0 = xpool.tile([128, DC, 512], BF16, name="xq_seg0", tag="xq")
        wq_sb = persist.tile([128, DC, FPC], BF16, name="wq_sb")
        wk_sb = persist.tile([128, DC, FPC], BF16, name="wk_sb")
        wv_sb = persist.tile([128, DC, FPC], BF16, name="wv_sb")
        wp_sb = persist.tile([128, 4, D], BF16, name="wp_sb")
        hc = DC // 2
        for h in range(2):
            cs = slice(512 * h, 512 * h + 512)
            nc.sync.dma_start(
                xq0[:, hc * h:hc * h + hc, :],
                xT_d[cs, 0:512].rearrange("(c p) s -> p c s", p=128))
            nc.sync.dma_start(
                wq_sb[:, hc * h:hc * h + hc, :],
                wq_d[cs, :].rearrange("(c p) f -> p c f", p=128))
        nc.sync.dma_start(wk_sb[:], wk_d.rearrange("(c p) f -> p c f", p=128))
        nc.sync.dma_start(wv_sb[:], wv_d.rearrange("(c p) f -> p c f", p=128))
        bq_sb = persist.tile([128, 4], F32, name="bq_sb")
        bk_sb = persist.tile([128, 4], F32, name="bk_sb")
        nc.sync.dma_start(bq_sb[:], bq_d.rearrange("(c p) -> p c", p=128))
        nc.sync.dma_start(bk_sb[:], bk_d.rearrange("(c p) -> p c", p=128))
        nc.sync.dma_start(wp_sb[:], wp_d.rearrange("(c p) f -> p c f", p=128))

        onesv = persist.tile([128, 8], BF16, name="onesv")
        nc.gpsimd.memset(onesv[:], 1.0)
        for i in range(NST):
            nc.vector.tensor_copy(Vt[i][:, :, 64], onesv[:])

        # ---- emission helpers ------------------------------------------
        def qk_units(seg, p, xq):
            """4 closures: Q(p) first/second half, K(p) first/second half."""
            s0 = 512 * seg
            st = {}

            def mk(nm, w_sb, b_sb, dstT):
                def u0():
                    ps = wps.tile([128, 512], F32, tag="wps",
                                  name=f"ps{nm}{seg}_{p}")
                    for c in range(4):
                        nc.tensor.matmul(ps[:], w_sb[:, c, 128 * p:128 * p + 128],
                                         xq[:, c, :], start=(c == 0), stop=False)
                    st[nm] = ps

                def u1():
                    ps = st[nm]
                    for c in range(4, DC):
                        nc.tensor.matmul(ps[:], w_sb[:, c, 128 * p:128 * p + 128],
                                         xq[:, c, :], start=False,
                                         stop=(c == DC - 1))
                    nc.vector.tensor_scalar_add(dstT[p][:, s0:s0 + 512], ps[:],
                                                b_sb[:, p:p + 1])
                return [u0, u1]

            return mk("q", wq_sb, bq_sb, QT) + mk("k", wk_sb, bk_sb, KT)

        def v_units(seg, xq):
            """4 closures, one V s-tile each."""
            us = []
            for ii in range(4):
                i = 4 * seg + ii

                def u(i=i, ii=ii):
                    ps = wps.tile([128, 512], F32, tag="wps", name=f"psv{i}")
                    for c in range(DC):
                        nc.tensor.matmul(ps[:], xq[:, c, 128 * ii:128 * ii + 128],
                                         wv_sb[:, c, :], start=(c == 0),
                                         stop=(c == DC - 1))
                    nc.vector.tensor_copy(
                        Vt[i][:, :, 0:64],
                        ps[:].rearrange("p (h u) -> p h u", h=8))
                us.append(u)
            return us

        def proj_units(j):
            """8 closures, one [128s, 512d] output tile each."""
            us = []
            for i4 in range(4):
                for o in range(2):
                    i = 4 * j + i4

                    def u(i=i, o=o):
                        po = wps.tile([128, 512], F32, tag="wps",
                                      name=f"po{i}_{o}")
                        for p2 in range(NPAIR):
                            nc.tensor.matmul(po[:],
                                             yT[p2][:, 128 * i:128 * i + 128],
                                             wp_sb[:, p2, 512 * o:512 * o + 512],
                                             start=(p2 == 0), stop=(p2 == 3))
                        ot = outsb.tile([128, 512], F32, tag="ot",
                                        name=f"ot{i}_{o}")
                        nc.vector.tensor_copy(ot[:], po[:])
                        nc.sync.dma_start(
                            out_d[128 * i:128 * i + 128, 512 * o:512 * o + 512],
                            ot[:])
                    us.append(u)
            return us

        def att_pair(j, p, inject):
            q0 = 512 * j
            nk = 4 * (j + 1)
            accA = accps.tile([65, 512], F32, tag="acc", name=f"accA{j}_{p}")
            accB = accps.tile([65, 512], F32, tag="acc", name=f"accB{j}_{p}")

            def emit_pv(t, ex, lo):
                nc.tensor.matmul(accA[:, lo:512], Vt[t][:, 2 * p, :],
                                 ex[:, lo:512], start=(t == 0),
                                 stop=(t == nk - 1))
                nc.tensor.matmul(accB[:, lo:512], Vt[t][:, 2 * p + 1, :],
                                 ex[:, 512 + lo:1024], start=(t == 0),
                                 stop=(t == nk - 1))

            pending = None
            for t in range(nk):
                k0 = 128 * t
                oi = t - 4 * j
                lo = max(0, 128 * oi)
                sc = scps.tile([128, 1024], F32, tag="sc", name=f"sc{j}_{p}_{t}")
                nc.tensor.matmul(sc[:, lo:512], KT[p][0:64, k0:k0 + 128],
                                 QT[p][0:64, q0 + lo:q0 + 512],
                                 start=True, stop=True)
                nc.tensor.matmul(sc[:, 512 + lo:1024], KT[p][64:128, k0:k0 + 128],
                                 QT[p][64:128, q0 + lo:q0 + 512],
                                 start=True, stop=True)
                ex = expool.tile([128, 1024], BF16, tag="ex",
                                 name=f"ex{j}_{p}_{t}")
                if oi < 0:
                    nc.scalar.activation(ex[:], sc[:], AF.Exp, scale=0.125)
                else:
                    if lo <= 256:
                        # one ACT op; the dead zone costs less than a 2nd
                        # op's fixed overhead at these widths
                        nc.scalar.activation(ex[:, lo:1024], sc[:, lo:1024],
                                             AF.Exp, scale=0.125)
                    else:
                        nc.scalar.activation(ex[:, lo:512], sc[:, lo:512],
                                             AF.Exp, scale=0.125)
                        nc.scalar.activation(ex[:, 512 + lo:1024],
                                             sc[:, 512 + lo:1024], AF.Exp,
                                             scale=0.125)
                    # strict upper triangle of the diagonal block
                    for lo2 in (lo, 512 + lo):
                        nc.gpsimd.affine_select(
                            out=ex[:, lo2:lo2 + 128], in_=ex[:, lo2:lo2 + 128],
                            compare_op=ALU.is_ge, fill=0.0,
                            base=0, pattern=[[1, 128]], channel_multiplier=-1)
                if pending is not None:
                    emit_pv(*pending)
                pending = (t, ex, lo)
                inject()
            emit_pv(*pending)

            # normalization: evacuate PSUM fast (recip + unnormalized copy),
            # then broadcast the reciprocal and scale yT in place — the PE
            # and the acc slots never wait on the broadcast
            denA = smpool.tile([1, 512], F32, tag="denA", name=f"denA{j}_{p}")
            denB = smpool.tile([1, 512], F32, tag="denB", name=f"denB{j}_{p}")
            recA = smpool.tile([1, 512], F32, tag="rec", name=f"recA{j}_{p}")
            recB = smpool.tile([1, 512], F32, tag="rec", name=f"recB{j}_{p}")
            bcA = smpool.tile([128, 512], F32, tag="bcA", name=f"bcA{j}_{p}")
            bcB = smpool.tile([128, 512], F32, tag="bcB", name=f"bcB{j}_{p}")
            for acc, den, rec, bc, hi in ((accA, denA, recA, bcA, 0),
                                          (accB, denB, recB, bcB, 1)):
                ys = yT[p][64 * hi:64 * hi + 64, q0:q0 + 512]
                nc.vector.tensor_copy(den[:], acc[64:65, :])
                nc.vector.reciprocal_approx_fast(rec[:], den[:])
                nc.vector.tensor_copy(ys, acc[0:64, :])
                nc.gpsimd.partition_broadcast(bc[:], rec[0:1, :], channels=128)
            # NB: must stay on DVE — gpsimd tensor_tensor needs the standard
            # ucode library while partition_broadcast needs attn, and each
            # library swap stalls gpsimd ~7us
            for bc, hi in ((bcA, 0), (bcB, 1)):
                ys = yT[p][64 * hi:64 * hi + 64, q0:q0 + 512]
                nc.vector.tensor_tensor(ys, ys,
                                        bc[64 * hi:64 * hi + 64, :], ALU.mult)

        # ---- main schedule ---------------------------------------------
        # Segment seg's attention stream absorbs, at sub-tile granularity:
        # this segment's remaining QK projections, the NEXT segment's full
        # QKV (so no PE-only stretches remain between segments), and the
        # PREVIOUS chunk's output projection.
        xqs = [xq0]
        for seg in range(1, 4):
            xqs.append(xpool.tile([128, DC, 512], BF16, name=f"xq{seg}",
                                  tag="xq"))

        for seg in range(4):
            if seg < 3:
                s1 = 512 * (seg + 1)
                nc.sync.dma_start(
                    xqs[seg + 1][:],
                    xT_d[:, s1:s1 + 512].rearrange("(c p) s -> p c s", p=128))
            xq = xqs[seg]
            if seg == 0:   # nothing earlier to hide these under
                for u in qk_units(0, 0, xq):
                    u()
                for u in v_units(0, xq):
                    u()
            queues = [[] for _ in range(NPAIR)]
            for pp in (1, 2, 3):
                queues[pp - 1] += qk_units(seg, pp, xq)
            pu = proj_units(seg - 1) if seg >= 1 else []
            nxt = []
            if seg < 3:
                nxt += v_units(seg + 1, xqs[seg + 1])
                nxt += qk_units(seg + 1, 0, xqs[seg + 1])
            queues[1] += pu[0:4] + nxt[0:2]
            queues[2] += pu[4:8] + nxt[2:5]
            queues[3] += nxt[5:8]
            for p in range(NPAIR):
                q = queues[p]

                def inject(q=q):
                    if q:
                        q.pop(0)()
                att_pair(seg, p, inject)
                while q:   # flush any leftovers at pair end
                    q.pop(0)()
        for u in proj_units(3):
            u()

    nc.compile()
    return nc


def _get_program():
    if "nc" not in _CACHE:
        _CACHE["nc"] = _build_program()
    return _CACHE["nc"]


def kernel(x, W_attn, b_attn, W_proj, b_proj, _trace=False, _trace_cores=None):
    x = np.asarray(x, np.float32)
    W_attn = np.asarray(W_attn, np.float32)
    b_attn = np.asarray(b_attn, np.float32)
    W_proj = np.asarray(W_proj, np.float32)
    b_proj = np.asarray(b_proj, np.float32)

    nc = _get_program()

    bf16 = ml_dtypes.bfloat16
    x16 = x.astype(bf16)
    Wa16 = W_attn.astype(bf16)
    Wp16 = W_proj.astype(bf16)

    in_maps = []
    for c in range(NCORES):
        b, g = divmod(c, 2)
        gc = slice(FPC * g, FPC * g + FPC)
        in_maps.append({
            "xT": np.ascontiguousarray(x16[b].T),
            "wq": np.ascontiguousarray(Wa16[:, 0 * D:1 * D][:, gc]),
            "wk": np.ascontiguousarray(Wa16[:, 1 * D:2 * D][:, gc]),
            "wv": np.ascontiguousarray(Wa16[:, 2 * D:3 * D][:, gc]),
            "wp": np.ascontiguousarray(Wp16[gc, :]),
            "bq": np.ascontiguousarray(b_attn[0 * D:1 * D][gc]),
            "bk": np.ascontiguousarray(b_attn[1 * D:2 * D][gc]),
        })

    kw = {}
    if _trace:
        kw = dict(trace=True, trace_cores=_trace_cores or [0])
    res = bass_utils.run_bass_kernel_spmd(nc, in_maps, core_ids=list(range(NCORES)),
                                          **kw)

    # host-side reduction: v-bias commutes through softmax -> fold via W_proj
    corr = b_proj + b_attn[2 * D:3 * D] @ W_proj
    out = np.empty((B, S, D), np.float32)
    for b in range(B):
        out[b] = res.results[2 * b]["out"] + res.results[2 * b + 1]["out"] + corr

    if _trace:
        kernel._last_results = res
    return out


# revision 24
# speedup vs baseline: 2.4027x; 1.0170x over previous
"""Causal self-attention Trainium2 Bass kernel (v3, bf16).

Problem (hardcoded): B=4, S=2048, D=1024, H=16 heads, head_dim=64.
    qkv = x @ W_attn + b_attn; causal softmax attention; y @ W_proj + b_proj.

Sharding over 8 NeuronCores: core c -> (batch b = c//2, head-group g = c%2).
Each core computes, for its batch and its 8 heads (512 feature dims):
    Q^T, K^T [512f, 2048s] and V [2048s, 512f] in bf16
    flash-style causal attention in transposed layout, per head:
        scores^T [128k, 512q] = K^T.T @ Q^T  (two heads concurrent via PE
        row groups 0/64), exp on ACT (bf16 out), causal mask for diagonal
        blocks via gpsimd affine_select, PV accumulation [65hd, 512q] with a
        ones column carrying the softmax denominator.
    normalization: DVE reciprocal of the denominator row, gpsimd
    partition_broadcast, DVE multiply writing bf16 y^T.
    projection: y^T.T @ W_proj -> [2048, 1024] fp32 partial.
All matmuls bf16 (separate LDWEIGHTS with FWL overlaps the previous matmul;
fp32r would self-load weights at ~180ns serialized per matmul).
QKV and projection matmuls are interleaved into the attention stream at
sub-tile granularity so the PE never idles while ACT exp catches up.
Host: out[b] = partial(core 2b) + partial(core 2b+1) + b_proj + b_attn_v @ W_proj.
"""
import sys
if '/opt/trn_rl_repo' not in sys.path:
    sys.path.insert(0, '/opt/trn_rl_repo')

import numpy as np
import ml_dtypes
import concourse.bass as bass
import concourse.mybir as mybir
import concourse.tile as tile
from concourse import bacc
from concourse import bass_utils
from concourse import library_config

F32 = mybir.dt.float32
BF16 = mybir.dt.bfloat16
AF = mybir.ActivationFunctionType
ALU = mybir.AluOpType

B, S, D, H, HD = 4, 2048, 1024, 16, 64
NCORES = 8
FPC = 512            # feature dims per core (8 heads * 64)
NPAIR = 4            # head pairs per core
DC = D // 128        # 8 contraction chunks
NST = S // 128       # 16 s-tiles

_CACHE = {}


def _build_program():
    nc = bacc.Bacc("TRN2", target_bir_lowering=False, debug=False,
                   enable_asserts=False, num_devices=NCORES)

    xT_d = nc.dram_tensor("xT", [D, S], BF16, kind="ExternalInput").ap()
    wq_d = nc.dram_tensor("wq", [D, FPC], BF16, kind="ExternalInput").ap()
    wk_d = nc.dram_tensor("wk", [D, FPC], BF16, kind="ExternalInput").ap()
    wv_d = nc.dram_tensor("wv", [D, FPC], BF16, kind="ExternalInput").ap()
    wp_d = nc.dram_tensor("wp", [FPC, D], BF16, kind="ExternalInput").ap()
    bq_d = nc.dram_tensor("bq", [FPC], F32, kind="ExternalInput").ap()
    bk_d = nc.dram_tensor("bk", [FPC], F32, kind="ExternalInput").ap()
    out_d = nc.dram_tensor("out", [S, D], F32, kind="ExternalOutput").ap()

    from contextlib import ExitStack
    with tile.TileContext(nc) as tc, ExitStack() as ctx:
        persist = ctx.enter_context(tc.tile_pool(name="persist", bufs=1))
        xpool = ctx.enter_context(tc.tile_pool(name="xpool", bufs=2))
        expool = ctx.enter_context(tc.tile_pool(name="expool", bufs=4))
        smpool = ctx.enter_context(tc.tile_pool(name="smpool", bufs=3))
        outsb = ctx.enter_context(tc.tile_pool(name="outsb", bufs=3))
        scps = ctx.enter_context(tc.tile_pool(name="scps", bufs=2, space="PSUM"))
        wps = ctx.enter_context(tc.tile_pool(name="wps", bufs=2, space="PSUM"))
        accps = ctx.enter_context(tc.tile_pool(name="accps", bufs=2, space="PSUM"))

        nc.gpsimd.load_library(library_config.attn)

        QT = [persist.tile([128, S], BF16, name=f"qt{p}") for p in range(NPAIR)]
        KT = [persist.tile([128, S], BF16, name=f"kt{p}") for p in range(NPAIR)]
        yT = [persist.tile([128, S], BF16, name=f"yt{p}") for p in range(NPAIR)]
        # V tiles: [128 s, 8 heads, 65] -- col 64 is the ones column (denominator)
        Vt = [persist.tile([128, 8, 65], BF16, name=f"v{i}") for i in range(NST)]

        # Inputs needed first (x chunk 0, W_q) are issued first in halves so
        # the first matmuls aren't stuck behind the full 5MB of input DMA
        # competing for HBM bandwidth.
        xq0 = xpool.tile([128, DC, 512], BF16, name="xq_seg0", tag="xq")
        wq_sb = persist.tile([128, DC, FPC], BF16, name="wq_sb")
        wk_sb = persist.tile([128, DC, FPC], BF16, name="wk_sb")
        wv_sb = persist.tile([128, DC, FPC], BF16, name="wv_sb")
        wp_sb = persist.tile([128, 4, D], BF16, name="wp_sb")
        qc = DC // 4
        for h in range(4):
            cs = slice(256 * h, 256 * h + 256)
            nc.sync.dma_start(
                xq0[:, qc * h:qc * h + qc, :],
                xT_d[cs, 0:512].rearrange("(c p) s -> p c s", p=128))
            nc.sync.dma_start(
                wq_sb[:, qc * h:qc * h + qc, :],
                wq_d[cs, :].rearrange("(c p) f -> p c f", p=128))
        nc.sync.dma_start(wk_sb[:], wk_d.rearrange("(c p) f -> p c f", p=128))
        nc.sync.dma_start(wv_sb[:], wv_d.rearrange("(c p) f -> p c f", p=128))
        bq_sb = persist.tile([128, 4], F32, name="bq_sb")
        bk_sb = persist.tile([128, 4], F32, name="bk_sb")
        nc.sync.dma_start(bq_sb[:], bq_d.rearrange("(c p) -> p c", p=128))
        nc.sync.dma_start(bk_sb[:], bk_d.rearrange("(c p) -> p c", p=128))
        nc.sync.dma_start(wp_sb[:], wp_d.rearrange("(c p) f -> p c f", p=128))

        onesv = persist.tile([128, 8], BF16, name="onesv")
        nc.gpsimd.memset(onesv[:], 1.0)
        for i in range(NST):
            nc.vector.tensor_copy(Vt[i][:, :, 64], onesv[:])

        # ---- emission helpers ------------------------------------------
        def qk_units(seg, p, xq):
            """4 closures: Q(p) first/second half, K(p) first/second half."""
            s0 = 512 * seg
            st = {}

            def mk(nm, w_sb, b_sb, dstT):
                def u0():
                    ps = wps.tile([128, 512], F32, tag="wps",
                                  name=f"ps{nm}{seg}_{p}")
                    for c in range(4):
                        nc.tensor.matmul(ps[:], w_sb[:, c, 128 * p:128 * p + 128],
                                         xq[:, c, :], start=(c == 0), stop=False)
                    st[nm] = ps

                def u1():
                    ps = st[nm]
                    for c in range(4, DC):
                        nc.tensor.matmul(ps[:], w_sb[:, c, 128 * p:128 * p + 128],
                                         xq[:, c, :], start=False,
                                         stop=(c == DC - 1))
                    # evacuate on ACT (bias-add rides free) to keep the DVE
                    # queue short — injected QKV units stall on DVE backlog
                    nc.scalar.activation(dstT[p][:, s0:s0 + 512], ps[:],
                                         AF.Identity, bias=b_sb[:, p:p + 1])
                return [u0, u1]

            return mk("q", wq_sb, bq_sb, QT) + mk("k", wk_sb, bk_sb, KT)

        def v_units(seg, xq):
            """4 closures, one V s-tile each."""
            us = []
            for ii in range(4):
                i = 4 * seg + ii

                def u(i=i, ii=ii):
                    ps = wps.tile([128, 512], F32, tag="wps", name=f"psv{i}")
                    for c in range(DC):
                        nc.tensor.matmul(ps[:], xq[:, c, 128 * ii:128 * ii + 128],
                                         wv_sb[:, c, :], start=(c == 0),
                                         stop=(c == DC - 1))
                    nc.vector.tensor_copy(
                        Vt[i][:, :, 0:64],
                        ps[:].rearrange("p (h u) -> p h u", h=8))
                us.append(u)
            return us

        def proj_units(j):
            """8 closures, one [128s, 512d] output tile each."""
            us = []
            for i4 in range(4):
                for o in range(2):
                    i = 4 * j + i4

                    def u(i=i, o=o):
                        po = wps.tile([128, 512], F32, tag="wps",
                                      name=f"po{i}_{o}")
                        for p2 in range(NPAIR):
                            nc.tensor.matmul(po[:],
                                             yT[p2][:, 128 * i:128 * i + 128],
                                             wp_sb[:, p2, 512 * o:512 * o + 512],
                                             start=(p2 == 0), stop=(p2 == 3))
                        ot = outsb.tile([128, 512], F32, tag="ot",
                                        name=f"ot{i}_{o}")
                        nc.vector.tensor_copy(ot[:], po[:])
                        nc.sync.dma_start(
                            out_d[128 * i:128 * i + 128, 512 * o:512 * o + 512],
                            ot[:])
                    us.append(u)
            return us

        def att_pair(j, p, inject):
            q0 = 512 * j
            nk = 4 * (j + 1)
            accA = accps.tile([65, 512], F32, tag="acc", name=f"accA{j}_{p}")
            accB = accps.tile([65, 512], F32, tag="acc", name=f"accB{j}_{p}")

            def emit_pv(t, ex, lo):
                nc.tensor.matmul(accA[:, lo:512], Vt[t][:, 2 * p, :],
                                 ex[:, lo:512], start=(t == 0),
                                 stop=(t == nk - 1))
                nc.tensor.matmul(accB[:, lo:512], Vt[t][:, 2 * p + 1, :],
                                 ex[:, 512 + lo:1024], start=(t == 0),
                                 stop=(t == nk - 1))

            pending = None
            for t in range(nk):
                k0 = 128 * t
                oi = t - 4 * j
                lo = max(0, 128 * oi)
                sc = scps.tile([128, 1024], F32, tag="sc", name=f"sc{j}_{p}_{t}")
                nc.tensor.matmul(sc[:, lo:512], KT[p][0:64, k0:k0 + 128],
                                 QT[p][0:64, q0 + lo:q0 + 512],
                                 start=True, stop=True)
                nc.tensor.matmul(sc[:, 512 + lo:1024], KT[p][64:128, k0:k0 + 128],
                                 QT[p][64:128, q0 + lo:q0 + 512],
                                 start=True, stop=True)
                ex = expool.tile([128, 1024], BF16, tag="ex",
                                 name=f"ex{j}_{p}_{t}")
                if oi < 0:
                    nc.scalar.activation(ex[:], sc[:], AF.Exp, scale=0.125)
                else:
                    if lo <= 256:
                        # one ACT op; the dead zone costs less than a 2nd
                        # op's fixed overhead at these widths
                        nc.scalar.activation(ex[:, lo:1024], sc[:, lo:1024],
                                             AF.Exp, scale=0.125)
                    else:
                        nc.scalar.activation(ex[:, lo:512], sc[:, lo:512],
                                             AF.Exp, scale=0.125)
                        nc.scalar.activation(ex[:, 512 + lo:1024],
                                             sc[:, 512 + lo:1024], AF.Exp,
                                             scale=0.125)
                    # strict upper triangle of the diagonal block
                    for lo2 in (lo, 512 + lo):
                        nc.gpsimd.affine_select(
                            out=ex[:, lo2:lo2 + 128], in_=ex[:, lo2:lo2 + 128],
                            compare_op=ALU.is_ge, fill=0.0,
                            base=0, pattern=[[1, 128]], channel_multiplier=-1)
                if pending is not None:
                    emit_pv(*pending)
                pending = (t, ex, lo)
                inject()
            emit_pv(*pending)

            # normalization: evacuate PSUM fast (recip + unnormalized copy),
            # then broadcast the reciprocal and scale yT in place — the PE
            # and the acc slots never wait on the broadcast
            denA = smpool.tile([1, 512], F32, tag="denA", name=f"denA{j}_{p}")
            denB = smpool.tile([1, 512], F32, tag="denB", name=f"denB{j}_{p}")
            recA = smpool.tile([1, 512], F32, tag="rec", name=f"recA{j}_{p}")
            recB = smpool.tile([1, 512], F32, tag="rec", name=f"recB{j}_{p}")
            bcA = smpool.tile([128, 512], F32, tag="bcA", name=f"bcA{j}_{p}")
            bcB = smpool.tile([128, 512], F32, tag="bcB", name=f"bcB{j}_{p}")
            for acc, den, rec, bc, hi in ((accA, denA, recA, bcA, 0),
                                          (accB, denB, recB, bcB, 1)):
                ys = yT[p][64 * hi:64 * hi + 64, q0:q0 + 512]
                nc.vector.tensor_copy(den[:], acc[64:65, :])
                nc.vector.reciprocal_approx_fast(rec[:], den[:])
                nc.vector.tensor_copy(ys, acc[0:64, :])
                nc.gpsimd.partition_broadcast(bc[:], rec[0:1, :], channels=128)
            # NB: must stay on DVE — gpsimd tensor_tensor needs the standard
            # ucode library while partition_broadcast needs attn, and each
            # library swap stalls gpsimd ~7us
            for bc, hi in ((bcA, 0), (bcB, 1)):
                ys = yT[p][64 * hi:64 * hi + 64, q0:q0 + 512]
                nc.vector.tensor_tensor(ys, ys,
                                        bc[64 * hi:64 * hi + 64, :], ALU.mult)

        # ---- main schedule ---------------------------------------------
        # Segment seg's attention stream absorbs, at sub-tile granularity:
        # this segment's remaining QK projections, the NEXT segment's full
        # QKV (so no PE-only stretches remain between segments), and the
        # PREVIOUS chunk's output projection.
        xqs = [xq0]
        for seg in range(1, 4):
            xqs.append(xpool.tile([128, DC, 512], BF16, name=f"xq{seg}",
                                  tag="xq"))

        for seg in range(4):
            if seg < 3:
                s1 = 512 * (seg + 1)
                nc.sync.dma_start(
                    xqs[seg + 1][:],
                    xT_d[:, s1:s1 + 512].rearrange("(c p) s -> p c s", p=128))
            xq = xqs[seg]
            if seg == 0:   # nothing earlier to hide these under
                for u in qk_units(0, 0, xq):
                    u()
                for u in v_units(0, xq):
                    u()
            queues = [[] for _ in range(NPAIR)]
            for pp in (1, 2, 3):
                queues[pp - 1] += qk_units(seg, pp, xq)
            pu = proj_units(seg - 1) if seg >= 1 else []
            nxt = []
            if seg < 3:
                nxt += v_units(seg + 1, xqs[seg + 1])
                nxt += qk_units(seg + 1, 0, xqs[seg + 1])
            queues[1] += pu[0:4] + nxt[0:2]
            queues[2] += pu[4:8] + nxt[2:5]
            queues[3] += nxt[5:8]
            for p in range(NPAIR):
                q = queues[p]

                def inject(q=q):
                    if q:
                        q.pop(0)()
                att_pair(seg, p, inject)
                while q:   # flush any leftovers at pair end
                    q.pop(0)()
        for u in proj_units(3):
            u()

    nc.compile()
    return nc


def _get_program():
    if "nc" not in _CACHE:
        _CACHE["nc"] = _build_program()
    return _CACHE["nc"]


def kernel(x, W_attn, b_attn, W_proj, b_proj, _trace=False, _trace_cores=None):
    x = np.asarray(x, np.float32)
    W_attn = np.asarray(W_attn, np.float32)
    b_attn = np.asarray(b_attn, np.float32)
    W_proj = np.asarray(W_proj, np.float32)
    b_proj = np.asarray(b_proj, np.float32)

    nc = _get_program()

    bf16 = ml_dtypes.bfloat16
    x16 = x.astype(bf16)
    Wa16 = W_attn.astype(bf16)
    Wp16 = W_proj.astype(bf16)

    in_maps = []
    for c in range(NCORES):
        b, g = divmod(c, 2)
        gc = slice(FPC * g, FPC * g + FPC)
        in_maps.append({
            "xT": np.ascontiguousarray(x16[b].T),
            "wq": np.ascontiguousarray(Wa16[:, 0 * D:1 * D][:, gc]),
            "wk": np.ascontiguousarray(Wa16[:, 1 * D:2 * D][:, gc]),
            "wv": np.ascontiguousarray(Wa16[:, 2 * D:3 * D][:, gc]),
            "wp": np.ascontiguousarray(Wp16[gc, :]),
            "bq": np.ascontiguousarray(b_attn[0 * D:1 * D][gc]),
            "bk": np.ascontiguousarray(b_attn[1 * D:2 * D][gc]),
        })

    kw = {}
    if _trace:
        kw = dict(trace=True, trace_cores=_trace_cores or [0])
    res = bass_utils.run_bass_kernel_spmd(nc, in_maps, core_ids=list(range(NCORES)),
                                          **kw)

    # host-side reduction: v-bias commutes through softmax -> fold via W_proj
    corr = b_proj + b_attn[2 * D:3 * D] @ W_proj
    out = np.empty((B, S, D), np.float32)
    for b in range(B):
        out[b] = res.results[2 * b]["out"] + res.results[2 * b + 1]["out"] + corr

    if _trace:
        kernel._last_results = res
    return out


# revision 28
# speedup vs baseline: 2.4244x; 1.0090x over previous
"""Causal self-attention Trainium2 Bass kernel (v3, bf16).

Problem (hardcoded): B=4, S=2048, D=1024, H=16 heads, head_dim=64.
    qkv = x @ W_attn + b_attn; causal softmax attention; y @ W_proj + b_proj.

Sharding over 8 NeuronCores: core c -> (batch b = c//2, head-group g = c%2).
Each core computes, for its batch and its 8 heads (512 feature dims):
    Q^T, K^T [512f, 2048s] and V [2048s, 512f] in bf16
    flash-style causal attention in transposed layout, per head:
        scores^T [128k, 512q] = K^T.T @ Q^T  (two heads concurrent via PE
        row groups 0/64), exp on ACT (bf16 out), causal mask for diagonal
        blocks via gpsimd affine_select, PV accumulation [65hd, 512q] with a
        ones column carrying the softmax denominator.
    normalization: DVE reciprocal of the denominator row, gpsimd
    partition_broadcast, DVE multiply writing bf16 y^T.
    projection: y^T.T @ W_proj -> [2048, 1024] fp32 partial.
All matmuls bf16 (separate LDWEIGHTS with FWL overlaps the previous matmul;
fp32r would self-load weights at ~180ns serialized per matmul).
QKV and projection matmuls are interleaved into the attention stream at
sub-tile granularity so the PE never idles while ACT exp catches up.
Host: out[b] = partial(core 2b) + partial(core 2b+1) + b_proj + b_attn_v @ W_proj.
"""
import sys
if '/opt/trn_rl_repo' not in sys.path:
    sys.path.insert(0, '/opt/trn_rl_repo')

import numpy as np
import ml_dtypes
import concourse.bass as bass
import concourse.mybir as mybir
import concourse.tile as tile
from concourse import bacc
from concourse import bass_utils
from concourse import library_config

F32 = mybir.dt.float32
BF16 = mybir.dt.bfloat16
AF = mybir.ActivationFunctionType
ALU = mybir.AluOpType

B, S, D, H, HD = 4, 2048, 1024, 16, 64
NCORES = 8
FPC = 512            # feature dims per core (8 heads * 64)
NPAIR = 4            # head pairs per core
DC = D // 128        # 8 contraction chunks
NST = S // 128       # 16 s-tiles

_CACHE = {}


def _build_program():
    nc = bacc.Bacc("TRN2", target_bir_lowering=False, debug=False,
                   enable_asserts=False, num_devices=NCORES)

    xT_d = nc.dram_tensor("xT", [D, S], BF16, kind="ExternalInput").ap()
    wq_d = nc.dram_tensor("wq", [D, FPC], BF16, kind="ExternalInput").ap()
    wk_d = nc.dram_tensor("wk", [D, FPC], BF16, kind="ExternalInput").ap()
    wv_d = nc.dram_tensor("wv", [D, FPC], BF16, kind="ExternalInput").ap()
    wp_d = nc.dram_tensor("wp", [FPC, D], BF16, kind="ExternalInput").ap()
    bq_d = nc.dram_tensor("bq", [FPC], F32, kind="ExternalInput").ap()
    bk_d = nc.dram_tensor("bk", [FPC], F32, kind="ExternalInput").ap()
    out_d = nc.dram_tensor("out", [S, D], F32, kind="ExternalOutput").ap()

    from contextlib import ExitStack
    with tile.TileContext(nc) as tc, ExitStack() as ctx:
        persist = ctx.enter_context(tc.tile_pool(name="persist", bufs=1))
        xpool = ctx.enter_context(tc.tile_pool(name="xpool", bufs=2))
        expool = ctx.enter_context(tc.tile_pool(name="expool", bufs=6))
        smpool = ctx.enter_context(tc.tile_pool(name="smpool", bufs=3))
        outsb = ctx.enter_context(tc.tile_pool(name="outsb", bufs=3))
        scps = ctx.enter_context(tc.tile_pool(name="scps", bufs=2, space="PSUM"))
        wps = ctx.enter_context(tc.tile_pool(name="wps", bufs=2, space="PSUM"))
        accps = ctx.enter_context(tc.tile_pool(name="accps", bufs=2, space="PSUM"))

        nc.gpsimd.load_library(library_config.attn)

        QT = [persist.tile([128, S], BF16, name=f"qt{p}") for p in range(NPAIR)]
        KT = [persist.tile([128, S], BF16, name=f"kt{p}") for p in range(NPAIR)]
        yT = [persist.tile([128, S], BF16, name=f"yt{p}") for p in range(NPAIR)]
        # V tiles: [128 s, 8 heads, 65] -- col 64 is the ones column (denominator)
        Vt = [persist.tile([128, 8, 65], BF16, name=f"v{i}") for i in range(NST)]

        # Inputs needed first (x chunk 0, W_q) are issued first in halves so
        # the first matmuls aren't stuck behind the full 5MB of input DMA
        # competing for HBM bandwidth.
        xq0 = xpool.tile([128, DC, 512], BF16, name="xq_seg0", tag="xq")
        wq_sb = persist.tile([128, DC, FPC], BF16, name="wq_sb")
        wk_sb = persist.tile([128, DC, FPC], BF16, name="wk_sb")
        wv_sb = persist.tile([128, DC, FPC], BF16, name="wv_sb")
        wp_sb = persist.tile([128, 4, D], BF16, name="wp_sb")
        qc = DC // 4
        for h in range(4):
            cs = slice(256 * h, 256 * h + 256)
            nc.sync.dma_start(
                xq0[:, qc * h:qc * h + qc, :],
                xT_d[cs, 0:512].rearrange("(c p) s -> p c s", p=128))
            nc.sync.dma_start(
                wq_sb[:, qc * h:qc * h + qc, :],
                wq_d[cs, :].rearrange("(c p) f -> p c f", p=128))
        nc.sync.dma_start(wk_sb[:], wk_d.rearrange("(c p) f -> p c f", p=128))
        nc.sync.dma_start(wv_sb[:], wv_d.rearrange("(c p) f -> p c f", p=128))
        bq_sb = persist.tile([128, 4], F32, name="bq_sb")
        bk_sb = persist.tile([128, 4], F32, name="bk_sb")
        nc.sync.dma_start(bq_sb[:], bq_d.rearrange("(c p) -> p c", p=128))
        nc.sync.dma_start(bk_sb[:], bk_d.rearrange("(c p) -> p c", p=128))
        nc.sync.dma_start(wp_sb[:], wp_d.rearrange("(c p) f -> p c f", p=128))

        onesv = persist.tile([128, 8], BF16, name="onesv")
        nc.gpsimd.memset(onesv[:], 1.0)
        for i in range(NST):
            nc.vector.tensor_copy(Vt[i][:, :, 64], onesv[:])

        # ---- emission helpers ------------------------------------------
        def qk_units(seg, p, xq):
            """4 closures: Q(p) first/second half, K(p) first/second half."""
            s0 = 512 * seg
            st = {}

            def mk(nm, w_sb, b_sb, dstT):
                def u0():
                    ps = wps.tile([128, 512], F32, tag="wps",
                                  name=f"ps{nm}{seg}_{p}")
                    for c in range(4):
                        nc.tensor.matmul(ps[:], w_sb[:, c, 128 * p:128 * p + 128],
                                         xq[:, c, :], start=(c == 0), stop=False)
                    st[nm] = ps

                def u1():
                    ps = st[nm]
                    for c in range(4, DC):
                        nc.tensor.matmul(ps[:], w_sb[:, c, 128 * p:128 * p + 128],
                                         xq[:, c, :], start=False,
                                         stop=(c == DC - 1))
                    # evacuate on ACT (bias-add rides free) to keep the DVE
                    # queue short — injected QKV units stall on DVE backlog
                    nc.scalar.activation(dstT[p][:, s0:s0 + 512], ps[:],
                                         AF.Identity, bias=b_sb[:, p:p + 1])
                return [u0, u1]

            return mk("q", wq_sb, bq_sb, QT) + mk("k", wk_sb, bk_sb, KT)

        def v_units(seg, xq):
            """4 closures, one V s-tile each."""
            us = []
            for ii in range(4):
                i = 4 * seg + ii

                def u(i=i, ii=ii):
                    ps = wps.tile([128, 512], F32, tag="wps", name=f"psv{i}")
                    for c in range(DC):
                        nc.tensor.matmul(ps[:], xq[:, c, 128 * ii:128 * ii + 128],
                                         wv_sb[:, c, :], start=(c == 0),
                                         stop=(c == DC - 1))
                    nc.vector.tensor_copy(
                        Vt[i][:, :, 0:64],
                        ps[:].rearrange("p (h u) -> p h u", h=8))
                us.append(u)
            return us

        def proj_units(j):
            """8 closures, one [128s, 512d] output tile each."""
            us = []
            for i4 in range(4):
                for o in range(2):
                    i = 4 * j + i4

                    def u(i=i, o=o):
                        po = wps.tile([128, 512], F32, tag="wps",
                                      name=f"po{i}_{o}")
                        for p2 in range(NPAIR):
                            nc.tensor.matmul(po[:],
                                             yT[p2][:, 128 * i:128 * i + 128],
                                             wp_sb[:, p2, 512 * o:512 * o + 512],
                                             start=(p2 == 0), stop=(p2 == 3))
                        ot = outsb.tile([128, 512], F32, tag="ot",
                                        name=f"ot{i}_{o}")
                        nc.vector.tensor_copy(ot[:], po[:])
                        nc.sync.dma_start(
                            out_d[128 * i:128 * i + 128, 512 * o:512 * o + 512],
                            ot[:])
                    us.append(u)
            return us

        def att_pair(j, p, inject):
            q0 = 512 * j
            nk = 4 * (j + 1)
            accA = accps.tile([65, 512], F32, tag="acc", name=f"accA{j}_{p}")
            accB = accps.tile([65, 512], F32, tag="acc", name=f"accB{j}_{p}")

            def emit_pv(t, ex, lo):
                nc.tensor.matmul(accA[:, lo:512], Vt[t][:, 2 * p, :],
                                 ex[:, lo:512], start=(t == 0),
                                 stop=(t == nk - 1))
                nc.tensor.matmul(accB[:, lo:512], Vt[t][:, 2 * p + 1, :],
                                 ex[:, 512 + lo:1024], start=(t == 0),
                                 stop=(t == nk - 1))

            pending = []
            for t in range(nk):
                k0 = 128 * t
                oi = t - 4 * j
                lo = max(0, 128 * oi)
                sc = scps.tile([128, 1024], F32, tag="sc", name=f"sc{j}_{p}_{t}")
                nc.tensor.matmul(sc[:, lo:512], KT[p][0:64, k0:k0 + 128],
                                 QT[p][0:64, q0 + lo:q0 + 512],
                                 start=True, stop=True)
                nc.tensor.matmul(sc[:, 512 + lo:1024], KT[p][64:128, k0:k0 + 128],
                                 QT[p][64:128, q0 + lo:q0 + 512],
                                 start=True, stop=True)
                ex = expool.tile([128, 1024], BF16, tag="ex",
                                 name=f"ex{j}_{p}_{t}")
                if oi < 0:
                    nc.scalar.activation(ex[:], sc[:], AF.Exp, scale=0.125)
                else:
                    if lo <= 256:
                        # one ACT op; the dead zone costs less than a 2nd
                        # op's fixed overhead at these widths
                        nc.scalar.activation(ex[:, lo:1024], sc[:, lo:1024],
                                             AF.Exp, scale=0.125)
                    else:
                        nc.scalar.activation(ex[:, lo:512], sc[:, lo:512],
                                             AF.Exp, scale=0.125)
                        nc.scalar.activation(ex[:, 512 + lo:1024],
                                             sc[:, 512 + lo:1024], AF.Exp,
                                             scale=0.125)
                    # strict upper triangle of the diagonal block
                    for lo2 in (lo, 512 + lo):
                        nc.gpsimd.affine_select(
                            out=ex[:, lo2:lo2 + 128], in_=ex[:, lo2:lo2 + 128],
                            compare_op=ALU.is_ge, fill=0.0,
                            base=0, pattern=[[1, 128]], channel_multiplier=-1)
                pending.append((t, ex, lo))
                if len(pending) > 2:   # 2-tile lag so PV never waits on exp
                    emit_pv(*pending.pop(0))
                inject()
            while pending:
                emit_pv(*pending.pop(0))

            # normalization: evacuate PSUM fast (recip + unnormalized copy),
            # then broadcast the reciprocal and scale yT in place — the PE
            # and the acc slots never wait on the broadcast
            denA = smpool.tile([1, 512], F32, tag="denA", name=f"denA{j}_{p}")
            denB = smpool.tile([1, 512], F32, tag="denB", name=f"denB{j}_{p}")
            recA = smpool.tile([1, 512], F32, tag="rec", name=f"recA{j}_{p}")
            recB = smpool.tile([1, 512], F32, tag="rec", name=f"recB{j}_{p}")
            bcA = smpool.tile([128, 512], F32, tag="bcA", name=f"bcA{j}_{p}")
            bcB = smpool.tile([128, 512], F32, tag="bcB", name=f"bcB{j}_{p}")
            last = (j == 3 and p == 3)
            for acc, den, rec, bc, hi in ((accA, denA, recA, bcA, 0),
                                          (accB, denB, recB, bcB, 1)):
                ys = yT[p][64 * hi:64 * hi + 64, q0:q0 + 512]
                nc.vector.tensor_copy(den[:], acc[64:65, :])
                nc.vector.reciprocal_approx_fast(rec[:], den[:])
                if not last:
                    nc.vector.tensor_copy(ys, acc[0:64, :])
                nc.gpsimd.partition_broadcast(bc[:], rec[0:1, :], channels=128)
            # NB: the multiplies must stay on DVE — gpsimd tensor_tensor needs
            # the standard ucode library while partition_broadcast needs attn,
            # and each library swap stalls gpsimd ~7us
            for acc, bc, hi in ((accA, bcA, 0), (accB, bcB, 1)):
                ys = yT[p][64 * hi:64 * hi + 64, q0:q0 + 512]
                if last:
                    # final pair feeds the tail projection: fuse copy+mult
                    # (PSUM operand) to shorten the chain before proj(3)
                    nc.vector.tensor_tensor(ys, acc[0:64, :],
                                            bc[64 * hi:64 * hi + 64, :],
                                            ALU.mult)
                else:
                    nc.vector.tensor_tensor(ys, ys,
                                            bc[64 * hi:64 * hi + 64, :],
                                            ALU.mult)

        # ---- main schedule ---------------------------------------------
        # Segment seg's attention stream absorbs, at sub-tile granularity:
        # this segment's remaining QK projections, the NEXT segment's full
        # QKV (so no PE-only stretches remain between segments), and the
        # PREVIOUS chunk's output projection.
        xqs = [xq0]
        for seg in range(1, 4):
            xqs.append(xpool.tile([128, DC, 512], BF16, name=f"xq{seg}",
                                  tag="xq"))

        for seg in range(4):
            if seg < 3:
                s1 = 512 * (seg + 1)
                nc.sync.dma_start(
                    xqs[seg + 1][:],
                    xT_d[:, s1:s1 + 512].rearrange("(c p) s -> p c s", p=128))
            xq = xqs[seg]
            if seg == 0:   # nothing earlier to hide these under
                for u in qk_units(0, 0, xq):
                    u()
                for u in v_units(0, xq):
                    u()
            queues = [[] for _ in range(NPAIR)]
            for pp in (1, 2, 3):
                queues[pp - 1] += qk_units(seg, pp, xq)
            pu = proj_units(seg - 1) if seg >= 1 else []
            nxt = []
            if seg < 3:
                nxt += v_units(seg + 1, xqs[seg + 1])
                nxt += qk_units(seg + 1, 0, xqs[seg + 1])
            queues[1] += pu[0:4] + nxt[0:2]
            queues[2] += pu[4:8] + nxt[2:5]
            queues[3] += nxt[5:8]
            for p in range(NPAIR):
                q = queues[p]

                def inject(q=q):
                    if q:
                        q.pop(0)()
                att_pair(seg, p, inject)
                while q:   # flush any leftovers at pair end
                    q.pop(0)()
        for u in proj_units(3):
            u()

    nc.compile()
    return nc


def _get_program():
    if "nc" not in _CACHE:
        _CACHE["nc"] = _build_program()
    return _CACHE["nc"]


def kernel(x, W_attn, b_attn, W_proj, b_proj, _trace=False, _trace_cores=None):
    x = np.asarray(x, np.float32)
    W_attn = np.asarray(W_attn, np.float32)
    b_attn = np.asarray(b_attn, np.float32)
    W_proj = np.asarray(W_proj, np.float32)
    b_proj = np.asarray(b_proj, np.float32)

    nc = _get_program()

    bf16 = ml_dtypes.bfloat16
    x16 = x.astype(bf16)
    Wa16 = W_attn.astype(bf16)
    Wp16 = W_proj.astype(bf16)

    in_maps = []
    for c in range(NCORES):
        b, g = divmod(c, 2)
        gc = slice(FPC * g, FPC * g + FPC)
        in_maps.append({
            "xT": np.ascontiguousarray(x16[b].T),
            "wq": np.ascontiguousarray(Wa16[:, 0 * D:1 * D][:, gc]),
            "wk": np.ascontiguousarray(Wa16[:, 1 * D:2 * D][:, gc]),
            "wv": np.ascontiguousarray(Wa16[:, 2 * D:3 * D][:, gc]),
            "wp": np.ascontiguousarray(Wp16[gc, :]),
            "bq": np.ascontiguousarray(b_attn[0 * D:1 * D][gc]),
            "bk": np.ascontiguousarray(b_attn[1 * D:2 * D][gc]),
        })

    kw = {}
    if _trace:
        kw = dict(trace=True, trace_cores=_trace_cores or [0])
    res = bass_utils.run_bass_kernel_spmd(nc, in_maps, core_ids=list(range(NCORES)),
                                          **kw)

    # host-side reduction: v-bias commutes through softmax -> fold via W_proj
    corr = b_proj + b_attn[2 * D:3 * D] @ W_proj
    out = np.empty((B, S, D), np.float32)
    for b in range(B):
        out[b] = res.results[2 * b]["out"] + res.results[2 * b + 1]["out"] + corr

    if _trace:
        kernel._last_results = res
    return out


# revision 32
# speedup vs baseline: 2.4293x; 1.0020x over previous
"""Causal self-attention Trainium2 Bass kernel (v3, bf16).

Problem (hardcoded): B=4, S=2048, D=1024, H=16 heads, head_dim=64.
    qkv = x @ W_attn + b_attn; causal softmax attention; y @ W_proj + b_proj.

Sharding over 8 NeuronCores: core c -> (batch b = c//2, head-group g = c%2).
Each core computes, for its batch and its 8 heads (512 feature dims):
    Q^T, K^T [512f, 2048s] and V [2048s, 512f] in bf16
    flash-style causal attention in transposed layout, per head:
        scores^T [128k, 512q] = K^T.T @ Q^T  (two heads concurrent via PE
        row groups 0/64), exp on ACT (bf16 out), causal mask for diagonal
        blocks via gpsimd affine_select, PV accumulation [65hd, 512q] with a
        ones column carrying the softmax denominator.
    normalization: DVE reciprocal of the denominator row, gpsimd
    partition_broadcast, DVE multiply writing bf16 y^T.
    projection: y^T.T @ W_proj -> [2048, 1024] fp32 partial.
All matmuls bf16 (separate LDWEIGHTS with FWL overlaps the previous matmul;
fp32r would self-load weights at ~180ns serialized per matmul).
QKV and projection matmuls are interleaved into the attention stream at
sub-tile granularity so the PE never idles while ACT exp catches up.
Host: out[b] = partial(core 2b) + partial(core 2b+1) + b_proj + b_attn_v @ W_proj.
"""
import sys
if '/opt/trn_rl_repo' not in sys.path:
    sys.path.insert(0, '/opt/trn_rl_repo')

import numpy as np
import ml_dtypes
import concourse.bass as bass
import concourse.mybir as mybir
import concourse.tile as tile
from concourse import bacc
from concourse import bass_utils
from concourse import library_config

F32 = mybir.dt.float32
BF16 = mybir.dt.bfloat16
AF = mybir.ActivationFunctionType
ALU = mybir.AluOpType

B, S, D, H, HD = 4, 2048, 1024, 16, 64
NCORES = 8
FPC = 512            # feature dims per core (8 heads * 64)
NPAIR = 4            # head pairs per core
DC = D // 128        # 8 contraction chunks
NST = S // 128       # 16 s-tiles

_CACHE = {}


def _build_program():
    nc = bacc.Bacc("TRN2", target_bir_lowering=False, debug=False,
                   enable_asserts=False, num_devices=NCORES)

    xT_d = nc.dram_tensor("xT", [D, S], BF16, kind="ExternalInput").ap()
    wq_d = nc.dram_tensor("wq", [D, FPC], BF16, kind="ExternalInput").ap()
    wk_d = nc.dram_tensor("wk", [D, FPC], BF16, kind="ExternalInput").ap()
    wv_d = nc.dram_tensor("wv", [D, FPC], BF16, kind="ExternalInput").ap()
    wp_d = nc.dram_tensor("wp", [FPC, D], BF16, kind="ExternalInput").ap()
    bq_d = nc.dram_tensor("bq", [FPC], F32, kind="ExternalInput").ap()
    bk_d = nc.dram_tensor("bk", [FPC], F32, kind="ExternalInput").ap()
    out_d = nc.dram_tensor("out", [S, D], F32, kind="ExternalOutput").ap()

    from contextlib import ExitStack
    with tile.TileContext(nc) as tc, ExitStack() as ctx:
        persist = ctx.enter_context(tc.tile_pool(name="persist", bufs=1))
        xpool = ctx.enter_context(tc.tile_pool(name="xpool", bufs=2))
        expool = ctx.enter_context(tc.tile_pool(name="expool", bufs=6))
        smpool = ctx.enter_context(tc.tile_pool(name="smpool", bufs=3))
        outsb = ctx.enter_context(tc.tile_pool(name="outsb", bufs=3))
        scps = ctx.enter_context(tc.tile_pool(name="scps", bufs=2, space="PSUM"))
        wps = ctx.enter_context(tc.tile_pool(name="wps", bufs=2, space="PSUM"))
        accps = ctx.enter_context(tc.tile_pool(name="accps", bufs=2, space="PSUM"))

        nc.gpsimd.load_library(library_config.attn)

        QT = [persist.tile([128, S], BF16, name=f"qt{p}") for p in range(NPAIR)]
        KT = [persist.tile([128, S], BF16, name=f"kt{p}") for p in range(NPAIR)]
        yT = [persist.tile([128, S], BF16, name=f"yt{p}") for p in range(NPAIR)]
        # V tiles: [128 s, 8 heads, 65] -- col 64 is the ones column (denominator)
        Vt = [persist.tile([128, 8, 65], BF16, name=f"v{i}") for i in range(NST)]

        # Inputs needed first (x chunk 0, W_q) are issued first in halves so
        # the first matmuls aren't stuck behind the full 5MB of input DMA
        # competing for HBM bandwidth.
        xq0 = xpool.tile([128, DC, 512], BF16, name="xq_seg0", tag="xq")
        wq_sb = persist.tile([128, DC, FPC], BF16, name="wq_sb")
        wk_sb = persist.tile([128, DC, FPC], BF16, name="wk_sb")
        wv_sb = persist.tile([128, DC, FPC], BF16, name="wv_sb")
        wp_sb = persist.tile([128, 4, D], BF16, name="wp_sb")
        qc = DC // 4
        for h in range(4):
            cs = slice(256 * h, 256 * h + 256)
            nc.sync.dma_start(
                xq0[:, qc * h:qc * h + qc, :],
                xT_d[cs, 0:512].rearrange("(c p) s -> p c s", p=128))
            nc.sync.dma_start(
                wq_sb[:, qc * h:qc * h + qc, :],
                wq_d[cs, :].rearrange("(c p) f -> p c f", p=128))
        nc.sync.dma_start(wk_sb[:], wk_d.rearrange("(c p) f -> p c f", p=128))
        nc.sync.dma_start(wv_sb[:], wv_d.rearrange("(c p) f -> p c f", p=128))
        bq_sb = persist.tile([128, 4], F32, name="bq_sb")
        bk_sb = persist.tile([128, 4], F32, name="bk_sb")
        nc.sync.dma_start(bq_sb[:], bq_d.rearrange("(c p) -> p c", p=128))
        nc.sync.dma_start(bk_sb[:], bk_d.rearrange("(c p) -> p c", p=128))
        nc.sync.dma_start(wp_sb[:], wp_d.rearrange("(c p) f -> p c f", p=128))

        # chunk-3 projection accumulates per-pair partials here (SBUF) so its
        # matmuls can inject into att(3) instead of serializing at the tail
        outacc = persist.tile([128, 8, 512], F32, name="outacc")

        onesv = persist.tile([128, 8], BF16, name="onesv")
        nc.gpsimd.memset(onesv[:], 1.0)
        for i in range(NST):
            nc.vector.tensor_copy(Vt[i][:, :, 64], onesv[:])

        # ---- emission helpers ------------------------------------------
        def qk_units(seg, p, xq):
            """4 closures: Q(p) first/second half, K(p) first/second half."""
            s0 = 512 * seg
            st = {}

            def mk(nm, w_sb, b_sb, dstT):
                def u0():
                    ps = wps.tile([128, 512], F32, tag="wps",
                                  name=f"ps{nm}{seg}_{p}")
                    for c in range(4):
                        nc.tensor.matmul(ps[:], w_sb[:, c, 128 * p:128 * p + 128],
                                         xq[:, c, :], start=(c == 0), stop=False)
                    st[nm] = ps

                def u1():
                    ps = st[nm]
                    for c in range(4, DC):
                        nc.tensor.matmul(ps[:], w_sb[:, c, 128 * p:128 * p + 128],
                                         xq[:, c, :], start=False,
                                         stop=(c == DC - 1))
                    # evacuate on ACT (bias-add rides free) to keep the DVE
                    # queue short — injected QKV units stall on DVE backlog
                    nc.scalar.activation(dstT[p][:, s0:s0 + 512], ps[:],
                                         AF.Identity, bias=b_sb[:, p:p + 1])
                return [u0, u1]

            return mk("q", wq_sb, bq_sb, QT) + mk("k", wk_sb, bk_sb, KT)

        def v_units(seg, xq):
            """4 closures, one V s-tile each."""
            us = []
            for ii in range(4):
                i = 4 * seg + ii

                def u(i=i, ii=ii):
                    ps = wps.tile([128, 512], F32, tag="wps", name=f"psv{i}")
                    for c in range(DC):
                        nc.tensor.matmul(ps[:], xq[:, c, 128 * ii:128 * ii + 128],
                                         wv_sb[:, c, :], start=(c == 0),
                                         stop=(c == DC - 1))
                    nc.vector.tensor_copy(
                        Vt[i][:, :, 0:64],
                        ps[:].rearrange("p (h u) -> p h u", h=8))
                us.append(u)
            return us

        def proj_units(j):
            """8 closures, one [128s, 512d] output tile each."""
            us = []
            for i4 in range(4):
                for o in range(2):
                    i = 4 * j + i4

                    def u(i=i, o=o):
                        po = wps.tile([128, 512], F32, tag="wps",
                                      name=f"po{i}_{o}")
                        for p2 in range(NPAIR):
                            nc.tensor.matmul(po[:],
                                             yT[p2][:, 128 * i:128 * i + 128],
                                             wp_sb[:, p2, 512 * o:512 * o + 512],
                                             start=(p2 == 0), stop=(p2 == 3))
                        ot = outsb.tile([128, 512], F32, tag="ot",
                                        name=f"ot{i}_{o}")
                        nc.vector.tensor_copy(ot[:], po[:])
                        nc.sync.dma_start(
                            out_d[128 * i:128 * i + 128, 512 * o:512 * o + 512],
                            ot[:])
                    us.append(u)
            return us

        def proj3_units(p):
            """Pair p's partial projection of chunk 3 into outacc."""
            us = []
            for i4 in range(4):
                for o in range(2):
                    i, k = 12 + i4, 2 * i4 + o

                    def u(i=i, o=o, k=k, p=p):
                        po = wps.tile([128, 512], F32, tag="wps",
                                      name=f"p3_{p}_{i}_{o}")
                        nc.tensor.matmul(po[:],
                                         yT[p][:, 128 * i:128 * i + 128],
                                         wp_sb[:, p, 512 * o:512 * o + 512],
                                         start=True, stop=True)
                        if p == 0:
                            nc.vector.tensor_copy(outacc[:, k, :], po[:])
                        else:
                            nc.vector.tensor_tensor(outacc[:, k, :], po[:],
                                                    outacc[:, k, :], ALU.add)
                        if p == 3:
                            nc.sync.dma_start(
                                out_d[128 * i:128 * i + 128,
                                      512 * o:512 * o + 512],
                                outacc[:, k, :])
                    us.append(u)
            return us

        def att_pair(j, p, inject):
            q0 = 512 * j
            nk = 4 * (j + 1)
            accA = accps.tile([65, 512], F32, tag="acc", name=f"accA{j}_{p}")
            accB = accps.tile([65, 512], F32, tag="acc", name=f"accB{j}_{p}")

            def emit_pv(t, ex, lo):
                nc.tensor.matmul(accA[:, lo:512], Vt[t][:, 2 * p, :],
                                 ex[:, lo:512], start=(t == 0),
                                 stop=(t == nk - 1))
                nc.tensor.matmul(accB[:, lo:512], Vt[t][:, 2 * p + 1, :],
                                 ex[:, 512 + lo:1024], start=(t == 0),
                                 stop=(t == nk - 1))

            pending = []
            for t in range(nk):
                k0 = 128 * t
                oi = t - 4 * j
                lo = max(0, 128 * oi)
                sc = scps.tile([128, 1024], F32, tag="sc", name=f"sc{j}_{p}_{t}")
                nc.tensor.matmul(sc[:, lo:512], KT[p][0:64, k0:k0 + 128],
                                 QT[p][0:64, q0 + lo:q0 + 512],
                                 start=True, stop=True)
                nc.tensor.matmul(sc[:, 512 + lo:1024], KT[p][64:128, k0:k0 + 128],
                                 QT[p][64:128, q0 + lo:q0 + 512],
                                 start=True, stop=True)
                ex = expool.tile([128, 1024], BF16, tag="ex",
                                 name=f"ex{j}_{p}_{t}")
                if oi < 0:
                    nc.scalar.activation(ex[:], sc[:], AF.Exp, scale=0.125)
                else:
                    if lo <= 256:
                        # one ACT op; the dead zone costs less than a 2nd
                        # op's fixed overhead at these widths
                        nc.scalar.activation(ex[:, lo:1024], sc[:, lo:1024],
                                             AF.Exp, scale=0.125)
                    else:
                        nc.scalar.activation(ex[:, lo:512], sc[:, lo:512],
                                             AF.Exp, scale=0.125)
                        nc.scalar.activation(ex[:, 512 + lo:1024],
                                             sc[:, 512 + lo:1024], AF.Exp,
                                             scale=0.125)
                    # strict upper triangle of the diagonal block
                    for lo2 in (lo, 512 + lo):
                        nc.gpsimd.affine_select(
                            out=ex[:, lo2:lo2 + 128], in_=ex[:, lo2:lo2 + 128],
                            compare_op=ALU.is_ge, fill=0.0,
                            base=0, pattern=[[1, 128]], channel_multiplier=-1)
                pending.append((t, ex, lo))
                if len(pending) > 2:   # 2-tile lag so PV never waits on exp
                    emit_pv(*pending.pop(0))
                inject()
            while pending:
                emit_pv(*pending.pop(0))

            # normalization: evacuate PSUM fast (recip + unnormalized copy),
            # then broadcast the reciprocal and scale yT in place — the PE
            # and the acc slots never wait on the broadcast
            denA = smpool.tile([1, 512], F32, tag="denA", name=f"denA{j}_{p}")
            denB = smpool.tile([1, 512], F32, tag="denB", name=f"denB{j}_{p}")
            recA = smpool.tile([1, 512], F32, tag="rec", name=f"recA{j}_{p}")
            recB = smpool.tile([1, 512], F32, tag="rec", name=f"recB{j}_{p}")
            bcA = smpool.tile([128, 512], F32, tag="bcA", name=f"bcA{j}_{p}")
            bcB = smpool.tile([128, 512], F32, tag="bcB", name=f"bcB{j}_{p}")
            last = (j == 3 and p == 3)
            for acc, den, rec, bc, hi in ((accA, denA, recA, bcA, 0),
                                          (accB, denB, recB, bcB, 1)):
                ys = yT[p][64 * hi:64 * hi + 64, q0:q0 + 512]
                nc.vector.tensor_copy(den[:], acc[64:65, :])
                nc.vector.reciprocal_approx_fast(rec[:], den[:])
                if not last:
                    nc.vector.tensor_copy(ys, acc[0:64, :])
                nc.gpsimd.partition_broadcast(bc[:], rec[0:1, :], channels=128)
            # NB: the multiplies must stay on DVE — gpsimd tensor_tensor needs
            # the standard ucode library while partition_broadcast needs attn,
            # and each library swap stalls gpsimd ~7us
            for acc, bc, hi in ((accA, bcA, 0), (accB, bcB, 1)):
                ys = yT[p][64 * hi:64 * hi + 64, q0:q0 + 512]
                if last:
                    # final pair feeds the tail projection: fuse copy+mult
                    # (PSUM operand) to shorten the chain before proj(3)
                    nc.vector.tensor_tensor(ys, acc[0:64, :],
                                            bc[64 * hi:64 * hi + 64, :],
                                            ALU.mult)
                else:
                    nc.vector.tensor_tensor(ys, ys,
                                            bc[64 * hi:64 * hi + 64, :],
                                            ALU.mult)

        # ---- main schedule ---------------------------------------------
        # Segment seg's attention stream absorbs, at sub-tile granularity:
        # this segment's remaining QK projections, the NEXT segment's full
        # QKV (so no PE-only stretches remain between segments), and the
        # PREVIOUS chunk's output projection.
        xqs = [xq0]
        for seg in range(1, 4):
            xqs.append(xpool.tile([128, DC, 512], BF16, name=f"xq{seg}",
                                  tag="xq"))

        for seg in range(4):
            if seg < 3:
                s1 = 512 * (seg + 1)
                nc.sync.dma_start(
                    xqs[seg + 1][:],
                    xT_d[:, s1:s1 + 512].rearrange("(c p) s -> p c s", p=128))
            xq = xqs[seg]
            if seg == 0:   # nothing earlier to hide these under
                for u in qk_units(0, 0, xq):
                    u()
                for u in v_units(0, xq):
                    u()
            queues = [[] for _ in range(NPAIR)]
            for pp in (1, 2, 3):
                queues[pp - 1] += qk_units(seg, pp, xq)
            pu = proj_units(seg - 1) if seg >= 1 else []
            nxt = []
            if seg < 3:
                nxt += v_units(seg + 1, xqs[seg + 1])
                nxt += qk_units(seg + 1, 0, xqs[seg + 1])
            queues[1] += pu[0:4] + nxt[0:2]
            queues[2] += pu[4:8] + nxt[2:5]
            queues[3] += nxt[5:8]
            if seg == 3:   # chunk-3 projection partials follow each pair
                for p in range(3):
                    queues[p + 1] += proj3_units(p)
            for p in range(NPAIR):
                q = queues[p]

                def inject(q=q):
                    if q:
                        q.pop(0)()
                att_pair(seg, p, inject)
                while q:   # flush any leftovers at pair end
                    q.pop(0)()
        for u in proj3_units(3):
            u()

    nc.compile()
    return nc


def _get_program():
    if "nc" not in _CACHE:
        _CACHE["nc"] = _build_program()
    return _CACHE["nc"]


def kernel(x, W_attn, b_attn, W_proj, b_proj, _trace=False, _trace_cores=None):
    x = np.asarray(x, np.float32)
    W_attn = np.asarray(W_attn, np.float32)
    b_attn = np.asarray(b_attn, np.float32)
    W_proj = np.asarray(W_proj, np.float32)
    b_proj = np.asarray(b_proj, np.float32)

    nc = _get_program()

    bf16 = ml_dtypes.bfloat16
    x16 = x.astype(bf16)
    Wa16 = W_attn.astype(bf16)
    Wp16 = W_proj.astype(bf16)

    in_maps = []
    for c in range(NCORES):
        b, g = divmod(c, 2)
        gc = slice(FPC * g, FPC * g + FPC)
        in_maps.append({
            "xT": np.ascontiguousarray(x16[b].T),
            "wq": np.ascontiguousarray(Wa16[:, 0 * D:1 * D][:, gc]),
            "wk": np.ascontiguousarray(Wa16[:, 1 * D:2 * D][:, gc]),
            "wv": np.ascontiguousarray(Wa16[:, 2 * D:3 * D][:, gc]),
            "wp": np.ascontiguousarray(Wp16[gc, :]),
            "bq": np.ascontiguousarray(b_attn[0 * D:1 * D][gc]),
            "bk": np.ascontiguousarray(b_attn[1 * D:2 * D][gc]),
        })

    kw = {}
    if _trace:
        kw = dict(trace=True, trace_cores=_trace_cores or [0])
    res = bass_utils.run_bass_kernel_spmd(nc, in_maps, core_ids=list(range(NCORES)),
                                          **kw)

    # host-side reduction: v-bias commutes through softmax -> fold via W_proj
    corr = b_proj + b_attn[2 * D:3 * D] @ W_proj
    out = np.empty((B, S, D), np.float32)
    for b in range(B):
        out[b] = res.results[2 * b]["out"] + res.results[2 * b + 1]["out"] + corr

    if _trace:
        kernel._last_results = res
    return out


# revision 36
# speedup vs baseline: 2.4443x; 1.0062x over previous
"""Causal self-attention Trainium2 Bass kernel (v3, bf16).

Problem (hardcoded): B=4, S=2048, D=1024, H=16 heads, head_dim=64.
    qkv = x @ W_attn + b_attn; causal softmax attention; y @ W_proj + b_proj.

Sharding over 8 NeuronCores: core c -> (batch b = c//2, head-group g = c%2).
Each core computes, for its batch and its 8 heads (512 feature dims):
    Q^T, K^T [512f, 2048s] and V [2048s, 512f] in bf16
    flash-style causal attention in transposed layout, per head:
        scores^T [128k, 512q] = K^T.T @ Q^T  (two heads concurrent via PE
        row groups 0/64), exp on ACT (bf16 out), causal mask for diagonal
        blocks via gpsimd affine_select, PV accumulation [65hd, 512q] with a
        ones column carrying the softmax denominator.
    normalization: DVE reciprocal of the denominator row, gpsimd
    partition_broadcast, DVE multiply writing bf16 y^T.
    projection: y^T.T @ W_proj -> [2048, 1024] fp32 partial.
All matmuls bf16 (separate LDWEIGHTS with FWL overlaps the previous matmul;
fp32r would self-load weights at ~180ns serialized per matmul).
QKV and projection matmuls are interleaved into the attention stream at
sub-tile granularity so the PE never idles while ACT exp catches up.
Host: out[b] = partial(core 2b) + partial(core 2b+1) + b_proj + b_attn_v @ W_proj.
"""
import sys
if '/opt/trn_rl_repo' not in sys.path:
    sys.path.insert(0, '/opt/trn_rl_repo')

import numpy as np
import ml_dtypes
import concourse.bass as bass
import concourse.mybir as mybir
import concourse.tile as tile
from concourse import bacc
from concourse import bass_utils
from concourse import library_config

F32 = mybir.dt.float32
BF16 = mybir.dt.bfloat16
AF = mybir.ActivationFunctionType
ALU = mybir.AluOpType

B, S, D, H, HD = 4, 2048, 1024, 16, 64
NCORES = 8
FPC = 512            # feature dims per core (8 heads * 64)
NPAIR = 4            # head pairs per core
DC = D // 128        # 8 contraction chunks
NST = S // 128       # 16 s-tiles

_CACHE = {}


def _build_program():
    nc = bacc.Bacc("TRN2", target_bir_lowering=False, debug=False,
                   enable_asserts=False, num_devices=NCORES)

    xT_d = nc.dram_tensor("xT", [D, S], BF16, kind="ExternalInput").ap()
    wq_d = nc.dram_tensor("wq", [D, FPC], BF16, kind="ExternalInput").ap()
    wk_d = nc.dram_tensor("wk", [D, FPC], BF16, kind="ExternalInput").ap()
    wv_d = nc.dram_tensor("wv", [D, FPC], BF16, kind="ExternalInput").ap()
    wp_d = nc.dram_tensor("wp", [FPC, D], BF16, kind="ExternalInput").ap()
    bq_d = nc.dram_tensor("bq", [FPC], F32, kind="ExternalInput").ap()
    bk_d = nc.dram_tensor("bk", [FPC], F32, kind="ExternalInput").ap()
    out_d = nc.dram_tensor("out", [S, D], F32, kind="ExternalOutput").ap()

    from contextlib import ExitStack
    with tile.TileContext(nc) as tc, ExitStack() as ctx:
        persist = ctx.enter_context(tc.tile_pool(name="persist", bufs=1))
        xpool = ctx.enter_context(tc.tile_pool(name="xpool", bufs=2))
        expool = ctx.enter_context(tc.tile_pool(name="expool", bufs=8))
        smpool = ctx.enter_context(tc.tile_pool(name="smpool", bufs=4))
        outsb = ctx.enter_context(tc.tile_pool(name="outsb", bufs=3))
        scps = ctx.enter_context(tc.tile_pool(name="scps", bufs=2, space="PSUM"))
        wps = ctx.enter_context(tc.tile_pool(name="wps", bufs=2, space="PSUM"))
        accps = ctx.enter_context(tc.tile_pool(name="accps", bufs=2, space="PSUM"))

        nc.gpsimd.load_library(library_config.attn)

        QT = [persist.tile([128, S], BF16, name=f"qt{p}") for p in range(NPAIR)]
        KT = [persist.tile([128, S], BF16, name=f"kt{p}") for p in range(NPAIR)]
        yT = [persist.tile([128, S], BF16, name=f"yt{p}") for p in range(NPAIR)]
        # V tiles: [128 s, 8 heads, 65] -- col 64 is the ones column (denominator)
        Vt = [persist.tile([128, 8, 65], BF16, name=f"v{i}") for i in range(NST)]

        # Inputs needed first (x chunk 0, W_q) are issued first in halves so
        # the first matmuls aren't stuck behind the full 5MB of input DMA
        # competing for HBM bandwidth.
        xq0 = xpool.tile([128, DC, 512], BF16, name="xq_seg0", tag="xq")
        wq_sb = persist.tile([128, DC, FPC], BF16, name="wq_sb")
        wk_sb = persist.tile([128, DC, FPC], BF16, name="wk_sb")
        wv_sb = persist.tile([128, DC, FPC], BF16, name="wv_sb")
        wp_sb = persist.tile([128, 4, D], BF16, name="wp_sb")
        qc = DC // 4
        for h in range(4):
            cs = slice(256 * h, 256 * h + 256)
            nc.sync.dma_start(
                xq0[:, qc * h:qc * h + qc, :],
                xT_d[cs, 0:512].rearrange("(c p) s -> p c s", p=128))
            nc.sync.dma_start(
                wq_sb[:, qc * h:qc * h + qc, :],
                wq_d[cs, :].rearrange("(c p) f -> p c f", p=128))
        nc.sync.dma_start(wk_sb[:], wk_d.rearrange("(c p) f -> p c f", p=128))
        nc.sync.dma_start(wv_sb[:], wv_d.rearrange("(c p) f -> p c f", p=128))
        bq_sb = persist.tile([128, 4], F32, name="bq_sb")
        bk_sb = persist.tile([128, 4], F32, name="bk_sb")
        nc.sync.dma_start(bq_sb[:], bq_d.rearrange("(c p) -> p c", p=128))
        nc.sync.dma_start(bk_sb[:], bk_d.rearrange("(c p) -> p c", p=128))

        # chunk-3 projection accumulates per-pair partials here (SBUF) so its
        # matmuls can inject into att(3) instead of serializing at the tail
        outacc = persist.tile([128, 8, 512], F32, name="outacc")

        onesv = persist.tile([128, 8], BF16, name="onesv")
        nc.gpsimd.memset(onesv[:], 1.0)
        for i in range(NST):
            nc.vector.tensor_copy(Vt[i][:, :, 64], onesv[:])

        # ---- emission helpers ------------------------------------------
        def qk_units(seg, p, xq):
            """4 closures: Q(p) first/second half, K(p) first/second half."""
            s0 = 512 * seg
            st = {}

            def mk(nm, w_sb, b_sb, dstT):
                def u0():
                    ps = wps.tile([128, 512], F32, tag="wps",
                                  name=f"ps{nm}{seg}_{p}")
                    for c in range(4):
                        nc.tensor.matmul(ps[:], w_sb[:, c, 128 * p:128 * p + 128],
                                         xq[:, c, :], start=(c == 0), stop=False)
                    st[nm] = ps

                def u1():
                    ps = st[nm]
                    for c in range(4, DC):
                        nc.tensor.matmul(ps[:], w_sb[:, c, 128 * p:128 * p + 128],
                                         xq[:, c, :], start=False,
                                         stop=(c == DC - 1))
                    # evacuate on ACT (bias-add rides free) to keep the DVE
                    # queue short — injected QKV units stall on DVE backlog
                    nc.scalar.activation(dstT[p][:, s0:s0 + 512], ps[:],
                                         AF.Identity, bias=b_sb[:, p:p + 1])
                return [u0, u1]

            return mk("q", wq_sb, bq_sb, QT) + mk("k", wk_sb, bk_sb, KT)

        def v_units(seg, xq):
            """4 closures, one V s-tile each."""
            us = []
            for ii in range(4):
                i = 4 * seg + ii

                def u(i=i, ii=ii):
                    ps = wps.tile([128, 512], F32, tag="wps", name=f"psv{i}")
                    for c in range(DC):
                        nc.tensor.matmul(ps[:], xq[:, c, 128 * ii:128 * ii + 128],
                                         wv_sb[:, c, :], start=(c == 0),
                                         stop=(c == DC - 1))
                    nc.vector.tensor_copy(
                        Vt[i][:, :, 0:64],
                        ps[:].rearrange("p (h u) -> p h u", h=8))
                us.append(u)
            return us

        def proj_units(j):
            """8 closures, one [128s, 512d] output tile each."""
            us = []
            for i4 in range(4):
                for o in range(2):
                    i = 4 * j + i4

                    def u(i=i, o=o):
                        po = wps.tile([128, 512], F32, tag="wps",
                                      name=f"po{i}_{o}")
                        for p2 in range(NPAIR):
                            nc.tensor.matmul(po[:],
                                             yT[p2][:, 128 * i:128 * i + 128],
                                             wp_sb[:, p2, 512 * o:512 * o + 512],
                                             start=(p2 == 0), stop=(p2 == 3))
                        ot = outsb.tile([128, 512], F32, tag="ot",
                                        name=f"ot{i}_{o}")
                        nc.vector.tensor_copy(ot[:], po[:])
                        nc.sync.dma_start(
                            out_d[128 * i:128 * i + 128, 512 * o:512 * o + 512],
                            ot[:])
                    us.append(u)
            return us

        def proj3_units(p):
            """Pair p's partial projection of chunk 3 into outacc."""
            us = []
            for i4 in range(4):
                for o in range(2):
                    i, k = 12 + i4, 2 * i4 + o

                    def u(i=i, o=o, k=k, p=p):
                        po = wps.tile([128, 512], F32, tag="wps",
                                      name=f"p3_{p}_{i}_{o}")
                        nc.tensor.matmul(po[:],
                                         yT[p][:, 128 * i:128 * i + 128],
                                         wp_sb[:, p, 512 * o:512 * o + 512],
                                         start=True, stop=True)
                        if p == 0:
                            nc.vector.tensor_copy(outacc[:, k, :], po[:])
                        else:
                            nc.vector.tensor_tensor(outacc[:, k, :], po[:],
                                                    outacc[:, k, :], ALU.add)
                        if p == 3:
                            nc.sync.dma_start(
                                out_d[128 * i:128 * i + 128,
                                      512 * o:512 * o + 512],
                                outacc[:, k, :])
                    us.append(u)
            return us

        def att_pair(j, p, inject):
            q0 = 512 * j
            nk = 4 * (j + 1)
            accA = accps.tile([65, 512], F32, tag="acc", name=f"accA{j}_{p}")
            accB = accps.tile([65, 512], F32, tag="acc", name=f"accB{j}_{p}")

            def emit_pv(t, ex, lo):
                nc.tensor.matmul(accA[:, lo:512], Vt[t][:, 2 * p, :],
                                 ex[:, lo:512], start=(t == 0),
                                 stop=(t == nk - 1))
                nc.tensor.matmul(accB[:, lo:512], Vt[t][:, 2 * p + 1, :],
                                 ex[:, 512 + lo:1024], start=(t == 0),
                                 stop=(t == nk - 1))

            pending = []
            for t in range(nk):
                k0 = 128 * t
                oi = t - 4 * j
                lo = max(0, 128 * oi)
                sc = scps.tile([128, 1024], F32, tag="sc", name=f"sc{j}_{p}_{t}")
                nc.tensor.matmul(sc[:, lo:512], KT[p][0:64, k0:k0 + 128],
                                 QT[p][0:64, q0 + lo:q0 + 512],
                                 start=True, stop=True)
                nc.tensor.matmul(sc[:, 512 + lo:1024], KT[p][64:128, k0:k0 + 128],
                                 QT[p][64:128, q0 + lo:q0 + 512],
                                 start=True, stop=True)
                ex = expool.tile([128, 1024], BF16, tag="ex",
                                 name=f"ex{j}_{p}_{t}")
                if oi < 0:
                    nc.scalar.activation(ex[:], sc[:], AF.Exp, scale=0.125)
                else:
                    if lo <= 256:
                        # one ACT op; the dead zone costs less than a 2nd
                        # op's fixed overhead at these widths
                        nc.scalar.activation(ex[:, lo:1024], sc[:, lo:1024],
                                             AF.Exp, scale=0.125)
                    else:
                        nc.scalar.activation(ex[:, lo:512], sc[:, lo:512],
                                             AF.Exp, scale=0.125)
                        nc.scalar.activation(ex[:, 512 + lo:1024],
                                             sc[:, 512 + lo:1024], AF.Exp,
                                             scale=0.125)
                    # strict upper triangle of the diagonal block
                    for lo2 in (lo, 512 + lo):
                        nc.gpsimd.affine_select(
                            out=ex[:, lo2:lo2 + 128], in_=ex[:, lo2:lo2 + 128],
                            compare_op=ALU.is_ge, fill=0.0,
                            base=0, pattern=[[1, 128]], channel_multiplier=-1)
                pending.append((t, ex, lo))
                if len(pending) > 3:   # 3-tile lag so PV never waits on exp
                    emit_pv(*pending.pop(0))
                inject()
            while pending:
                emit_pv(*pending.pop(0))

            # normalization: evacuate PSUM fast (recip + unnormalized copy),
            # then broadcast the reciprocal and scale yT in place — the PE
            # and the acc slots never wait on the broadcast
            denA = smpool.tile([1, 512], F32, tag="denA", name=f"denA{j}_{p}")
            denB = smpool.tile([1, 512], F32, tag="denB", name=f"denB{j}_{p}")
            recA = smpool.tile([1, 512], F32, tag="rec", name=f"recA{j}_{p}")
            recB = smpool.tile([1, 512], F32, tag="rec", name=f"recB{j}_{p}")
            bcA = smpool.tile([128, 512], F32, tag="bcA", name=f"bcA{j}_{p}")
            bcB = smpool.tile([128, 512], F32, tag="bcB", name=f"bcB{j}_{p}")
            last = (j == 3 and p == 3)
            for acc, den, rec, bc, hi in ((accA, denA, recA, bcA, 0),
                                          (accB, denB, recB, bcB, 1)):
                ys = yT[p][64 * hi:64 * hi + 64, q0:q0 + 512]
                nc.vector.tensor_copy(den[:], acc[64:65, :])
                nc.vector.reciprocal_approx_fast(rec[:], den[:])
                if not last:
                    nc.vector.tensor_copy(ys, acc[0:64, :])
                nc.gpsimd.partition_broadcast(bc[:], rec[0:1, :], channels=128)
            # NB: the multiplies must stay on DVE — gpsimd tensor_tensor needs
            # the standard ucode library while partition_broadcast needs attn,
            # and each library swap stalls gpsimd ~7us
            for acc, bc, hi in ((accA, bcA, 0), (accB, bcB, 1)):
                ys = yT[p][64 * hi:64 * hi + 64, q0:q0 + 512]
                if last:
                    # final pair feeds the tail projection: fuse copy+mult
                    # (PSUM operand) to shorten the chain before proj(3)
                    nc.vector.tensor_tensor(ys, acc[0:64, :],
                                            bc[64 * hi:64 * hi + 64, :],
                                            ALU.mult)
                else:
                    nc.vector.tensor_tensor(ys, ys,
                                            bc[64 * hi:64 * hi + 64, :],
                                            ALU.mult)

        # ---- main schedule ---------------------------------------------
        # Segment seg's attention stream absorbs, at sub-tile granularity:
        # this segment's remaining QK projections, the NEXT segment's full
        # QKV (so no PE-only stretches remain between segments), and the
        # PREVIOUS chunk's output projection.
        xqs = [xq0]
        for seg in range(1, 4):
            xqs.append(xpool.tile([128, DC, 512], BF16, name=f"xq{seg}",
                                  tag="xq"))

        for seg in range(4):
            if seg < 3:
                s1 = 512 * (seg + 1)
                nc.sync.dma_start(
                    xqs[seg + 1][:],
                    xT_d[:, s1:s1 + 512].rearrange("(c p) s -> p c s", p=128))
            if seg == 0:   # W_proj isn't read until proj(0) during att(1)
                nc.sync.dma_start(wp_sb[:],
                                  wp_d.rearrange("(c p) f -> p c f", p=128))
            xq = xqs[seg]
            if seg == 0:   # nothing earlier to hide these under
                for u in qk_units(0, 0, xq):
                    u()
                for u in v_units(0, xq):
                    u()
            queues = [[] for _ in range(NPAIR)]
            for pp in (1, 2, 3):
                queues[pp - 1] += qk_units(seg, pp, xq)
            pu = proj_units(seg - 1) if seg >= 1 else []
            nxt = []
            if seg < 3:
                nxt += v_units(seg + 1, xqs[seg + 1])
                nxt += qk_units(seg + 1, 0, xqs[seg + 1])
            queues[1] += pu[0:4] + nxt[0:2]
            queues[2] += pu[4:8] + nxt[2:5]
            queues[3] += nxt[5:8]
            if seg == 3:   # chunk-3 projection partials follow each pair
                for p in range(3):
                    queues[p + 1] += proj3_units(p)
            for p in range(NPAIR):
                q = queues[p]

                def inject(q=q):
                    if q:
                        q.pop(0)()
                att_pair(seg, p, inject)
                while q:   # flush any leftovers at pair end
                    q.pop(0)()
        for u in proj3_units(3):
            u()

    nc.compile()
    return nc


def _get_program():
    if "nc" not in _CACHE:
        _CACHE["nc"] = _build_program()
    return _CACHE["nc"]


def kernel(x, W_attn, b_attn, W_proj, b_proj, _trace=False, _trace_cores=None):
    x = np.asarray(x, np.float32)
    W_attn = np.asarray(W_attn, np.float32)
    b_attn = np.asarray(b_attn, np.float32)
    W_proj = np.asarray(W_proj, np.float32)
    b_proj = np.asarray(b_proj, np.float32)

    nc = _get_program()

    bf16 = ml_dtypes.bfloat16
    x16 = x.astype(bf16)
    Wa16 = W_attn.astype(bf16)
    Wp16 = W_proj.astype(bf16)

    in_maps = []
    for c in range(NCORES):
        b, g = divmod(c, 2)
        gc = slice(FPC * g, FPC * g + FPC)
        in_maps.append({
            "xT": np.ascontiguousarray(x16[b].T),
            "wq": np.ascontiguousarray(Wa16[:, 0 * D:1 * D][:, gc]),
            "wk": np.ascontiguousarray(Wa16[:, 1 * D:2 * D][:, gc]),
            "wv": np.ascontiguousarray(Wa16[:, 2 * D:3 * D][:, gc]),
            "wp": np.ascontiguousarray(Wp16[gc, :]),
            "bq": np.ascontiguousarray(b_attn[0 * D:1 * D][gc]),
            "bk": np.ascontiguousarray(b_attn[1 * D:2 * D][gc]),
        })

    kw = {}
    if _trace:
        kw = dict(trace=True, trace_cores=_trace_cores or [0])
    res = bass_utils.run_bass_kernel_spmd(nc, in_maps, core_ids=list(range(NCORES)),
                                          **kw)

    # host-side reduction: v-bias commutes through softmax -> fold via W_proj
    corr = b_proj + b_attn[2 * D:3 * D] @ W_proj
    out = np.empty((B, S, D), np.float32)
    for b in range(B):
        out[b] = res.results[2 * b]["out"] + res.results[2 * b + 1]["out"] + corr

    if _trace:
        kernel._last_results = res
    return out


# revision 37
# speedup vs baseline: 2.4825x; 1.0156x over previous
"""Causal self-attention Trainium2 Bass kernel (v3, bf16).

Problem (hardcoded): B=4, S=2048, D=1024, H=16 heads, head_dim=64.
    qkv = x @ W_attn + b_attn; causal softmax attention; y @ W_proj + b_proj.

Sharding over 8 NeuronCores: core c -> (batch b = c//2, head-group g = c%2).
Each core computes, for its batch and its 8 heads (512 feature dims):
    Q^T, K^T [512f, 2048s] and V [2048s, 512f] in bf16
    flash-style causal attention in transposed layout, per head:
        scores^T [128k, 512q] = K^T.T @ Q^T  (two heads concurrent via PE
        row groups 0/64), exp on ACT (bf16 out), causal mask for diagonal
        blocks via gpsimd affine_select, PV accumulation [65hd, 512q] with a
        ones column carrying the softmax denominator.
    normalization: DVE reciprocal of the denominator row, gpsimd
    partition_broadcast, DVE multiply writing bf16 y^T.
    projection: y^T.T @ W_proj -> [2048, 1024] fp32 partial.
All matmuls bf16 (separate LDWEIGHTS with FWL overlaps the previous matmul;
fp32r would self-load weights at ~180ns serialized per matmul).
QKV and projection matmuls are interleaved into the attention stream at
sub-tile granularity so the PE never idles while ACT exp catches up.
Host: out[b] = partial(core 2b) + partial(core 2b+1) + b_proj + b_attn_v @ W_proj.
"""
import sys
if '/opt/trn_rl_repo' not in sys.path:
    sys.path.insert(0, '/opt/trn_rl_repo')

import numpy as np
import ml_dtypes
import concourse.bass as bass
import concourse.mybir as mybir
import concourse.tile as tile
from concourse import bacc
from concourse import bass_utils
from concourse import library_config

F32 = mybir.dt.float32
BF16 = mybir.dt.bfloat16
AF = mybir.ActivationFunctionType
ALU = mybir.AluOpType

B, S, D, H, HD = 4, 2048, 1024, 16, 64
NCORES = 8
FPC = 512            # feature dims per core (8 heads * 64)
NPAIR = 4            # head pairs per core
DC = D // 128        # 8 contraction chunks
NST = S // 128       # 16 s-tiles

_CACHE = {}


def _build_program():
    nc = bacc.Bacc("TRN2", target_bir_lowering=False, debug=False,
                   enable_asserts=False, num_devices=NCORES)

    xT_d = nc.dram_tensor("xT", [D, S], BF16, kind="ExternalInput").ap()
    wq_d = nc.dram_tensor("wq", [D, FPC], BF16, kind="ExternalInput").ap()
    wk_d = nc.dram_tensor("wk", [D, FPC], BF16, kind="ExternalInput").ap()
    wv_d = nc.dram_tensor("wv", [D, FPC], BF16, kind="ExternalInput").ap()
    wp_d = nc.dram_tensor("wp", [FPC, D], BF16, kind="ExternalInput").ap()
    bq_d = nc.dram_tensor("bq", [FPC], F32, kind="ExternalInput").ap()
    bk_d = nc.dram_tensor("bk", [FPC], F32, kind="ExternalInput").ap()
    out_d = nc.dram_tensor("out", [S, D], F32, kind="ExternalOutput").ap()

    from contextlib import ExitStack
    with tile.TileContext(nc) as tc, ExitStack() as ctx:
        persist = ctx.enter_context(tc.tile_pool(name="persist", bufs=1))
        xpool = ctx.enter_context(tc.tile_pool(name="xpool", bufs=2))
        expool = ctx.enter_context(tc.tile_pool(name="expool", bufs=8))
        smpool = ctx.enter_context(tc.tile_pool(name="smpool", bufs=4))
        outsb = ctx.enter_context(tc.tile_pool(name="outsb", bufs=3))
        scps = ctx.enter_context(tc.tile_pool(name="scps", bufs=2, space="PSUM"))
        wps = ctx.enter_context(tc.tile_pool(name="wps", bufs=2, space="PSUM"))
        accps = ctx.enter_context(tc.tile_pool(name="accps", bufs=2, space="PSUM"))

        nc.gpsimd.load_library(library_config.attn)

        QT = [persist.tile([128, S], BF16, name=f"qt{p}") for p in range(NPAIR)]
        KT = [persist.tile([128, S], BF16, name=f"kt{p}") for p in range(NPAIR)]
        yT = [persist.tile([128, S], BF16, name=f"yt{p}") for p in range(NPAIR)]
        # V tiles: [128 s, 8 heads, 65] -- col 64 is the ones column (denominator)
        Vt = [persist.tile([128, 8, 65], BF16, name=f"v{i}") for i in range(NST)]

        # Inputs needed first (x chunk 0, W_q) are issued first in halves so
        # the first matmuls aren't stuck behind the full 5MB of input DMA
        # competing for HBM bandwidth.
        xq0 = xpool.tile([128, DC, 512], BF16, name="xq_seg0", tag="xq")
        wq_sb = persist.tile([128, DC, FPC], BF16, name="wq_sb")
        wk_sb = persist.tile([128, DC, FPC], BF16, name="wk_sb")
        wv_sb = persist.tile([128, DC, FPC], BF16, name="wv_sb")
        wp_sb = persist.tile([128, 4, D], BF16, name="wp_sb")
        qc = DC // 4
        for h in range(4):
            cs = slice(256 * h, 256 * h + 256)
            nc.sync.dma_start(
                xq0[:, qc * h:qc * h + qc, :],
                xT_d[cs, 0:512].rearrange("(c p) s -> p c s", p=128))
            nc.sync.dma_start(
                wq_sb[:, qc * h:qc * h + qc, :],
                wq_d[cs, :].rearrange("(c p) f -> p c f", p=128))
        nc.sync.dma_start(wk_sb[:], wk_d.rearrange("(c p) f -> p c f", p=128))
        nc.sync.dma_start(wv_sb[:], wv_d.rearrange("(c p) f -> p c f", p=128))
        bq_sb = persist.tile([128, 4], F32, name="bq_sb")
        bk_sb = persist.tile([128, 4], F32, name="bk_sb")
        nc.sync.dma_start(bq_sb[:], bq_d.rearrange("(c p) -> p c", p=128))
        nc.sync.dma_start(bk_sb[:], bk_d.rearrange("(c p) -> p c", p=128))

        # chunk-3 projection accumulates per-pair partials here (SBUF) so its
        # matmuls can inject into att(3) instead of serializing at the tail
        outacc = persist.tile([128, 8, 512], F32, name="outacc")

        onesv = persist.tile([128, 8], BF16, name="onesv")
        nc.gpsimd.memset(onesv[:], 1.0)
        for i in range(NST):
            nc.vector.tensor_copy(Vt[i][:, :, 64], onesv[:])

        # ---- emission helpers ------------------------------------------
        def qk_units(seg, p, xq):
            """4 closures: Q(p) first/second half, K(p) first/second half."""
            s0 = 512 * seg
            st = {}

            def mk(nm, w_sb, b_sb, dstT):
                def u0():
                    ps = wps.tile([128, 512], F32, tag="wps",
                                  name=f"ps{nm}{seg}_{p}")
                    for c in range(4):
                        nc.tensor.matmul(ps[:], w_sb[:, c, 128 * p:128 * p + 128],
                                         xq[:, c, :], start=(c == 0), stop=False)
                    st[nm] = ps

                def u1():
                    ps = st[nm]
                    for c in range(4, DC):
                        nc.tensor.matmul(ps[:], w_sb[:, c, 128 * p:128 * p + 128],
                                         xq[:, c, :], start=False,
                                         stop=(c == DC - 1))
                    # evacuate on ACT (bias-add rides free) to keep the DVE
                    # queue short — injected QKV units stall on DVE backlog
                    nc.scalar.activation(dstT[p][:, s0:s0 + 512], ps[:],
                                         AF.Identity, bias=b_sb[:, p:p + 1])
                return [u0, u1]

            return mk("q", wq_sb, bq_sb, QT) + mk("k", wk_sb, bk_sb, KT)

        def v_units(seg, xq):
            """4 closures, one V s-tile each."""
            us = []
            for ii in range(4):
                i = 4 * seg + ii

                def u(i=i, ii=ii):
                    ps = wps.tile([128, 512], F32, tag="wps", name=f"psv{i}")
                    for c in range(DC):
                        nc.tensor.matmul(ps[:], xq[:, c, 128 * ii:128 * ii + 128],
                                         wv_sb[:, c, :], start=(c == 0),
                                         stop=(c == DC - 1))
                    nc.scalar.activation(
                        Vt[i][:, :, 0:64],
                        ps[:].rearrange("p (h u) -> p h u", h=8), AF.Identity)
                us.append(u)
            return us

        def proj_units(j):
            """8 closures, one [128s, 512d] output tile each."""
            us = []
            for i4 in range(4):
                for o in range(2):
                    i = 4 * j + i4

                    def u(i=i, o=o):
                        po = wps.tile([128, 512], F32, tag="wps",
                                      name=f"po{i}_{o}")
                        for p2 in range(NPAIR):
                            nc.tensor.matmul(po[:],
                                             yT[p2][:, 128 * i:128 * i + 128],
                                             wp_sb[:, p2, 512 * o:512 * o + 512],
                                             start=(p2 == 0), stop=(p2 == 3))
                        ot = outsb.tile([128, 512], F32, tag="ot",
                                        name=f"ot{i}_{o}")
                        nc.vector.tensor_copy(ot[:], po[:])
                        nc.sync.dma_start(
                            out_d[128 * i:128 * i + 128, 512 * o:512 * o + 512],
                            ot[:])
                    us.append(u)
            return us

        def proj3_units(p):
            """Pair p's partial projection of chunk 3 into outacc."""
            us = []
            for i4 in range(4):
                for o in range(2):
                    i, k = 12 + i4, 2 * i4 + o

                    def u(i=i, o=o, k=k, p=p):
                        po = wps.tile([128, 512], F32, tag="wps",
                                      name=f"p3_{p}_{i}_{o}")
                        nc.tensor.matmul(po[:],
                                         yT[p][:, 128 * i:128 * i + 128],
                                         wp_sb[:, p, 512 * o:512 * o + 512],
                                         start=True, stop=True)
                        if p == 0:
                            nc.vector.tensor_copy(outacc[:, k, :], po[:])
                        else:
                            nc.vector.tensor_tensor(outacc[:, k, :], po[:],
                                                    outacc[:, k, :], ALU.add)
                        if p == 3:
                            nc.sync.dma_start(
                                out_d[128 * i:128 * i + 128,
                                      512 * o:512 * o + 512],
                                outacc[:, k, :])
                    us.append(u)
            return us

        def att_pair(j, p, inject):
            q0 = 512 * j
            nk = 4 * (j + 1)
            accA = accps.tile([65, 512], F32, tag="acc", name=f"accA{j}_{p}")
            accB = accps.tile([65, 512], F32, tag="acc", name=f"accB{j}_{p}")

            def emit_pv(t, ex, lo):
                nc.tensor.matmul(accA[:, lo:512], Vt[t][:, 2 * p, :],
                                 ex[:, lo:512], start=(t == 0),
                                 stop=(t == nk - 1))
                nc.tensor.matmul(accB[:, lo:512], Vt[t][:, 2 * p + 1, :],
                                 ex[:, 512 + lo:1024], start=(t == 0),
                                 stop=(t == nk - 1))

            pending = []
            for t in range(nk):
                k0 = 128 * t
                oi = t - 4 * j
                lo = max(0, 128 * oi)
                sc = scps.tile([128, 1024], F32, tag="sc", name=f"sc{j}_{p}_{t}")
                nc.tensor.matmul(sc[:, lo:512], KT[p][0:64, k0:k0 + 128],
                                 QT[p][0:64, q0 + lo:q0 + 512],
                                 start=True, stop=True)
                nc.tensor.matmul(sc[:, 512 + lo:1024], KT[p][64:128, k0:k0 + 128],
                                 QT[p][64:128, q0 + lo:q0 + 512],
                                 start=True, stop=True)
                ex = expool.tile([128, 1024], BF16, tag="ex",
                                 name=f"ex{j}_{p}_{t}")
                if oi < 0:
                    nc.scalar.activation(ex[:], sc[:], AF.Exp, scale=0.125)
                else:
                    if lo <= 256:
                        # one ACT op; the dead zone costs less than a 2nd
                        # op's fixed overhead at these widths
                        nc.scalar.activation(ex[:, lo:1024], sc[:, lo:1024],
                                             AF.Exp, scale=0.125)
                    else:
                        nc.scalar.activation(ex[:, lo:512], sc[:, lo:512],
                                             AF.Exp, scale=0.125)
                        nc.scalar.activation(ex[:, 512 + lo:1024],
                                             sc[:, 512 + lo:1024], AF.Exp,
                                             scale=0.125)
                    # strict upper triangle of the diagonal block
                    for lo2 in (lo, 512 + lo):
                        nc.gpsimd.affine_select(
                            out=ex[:, lo2:lo2 + 128], in_=ex[:, lo2:lo2 + 128],
                            compare_op=ALU.is_ge, fill=0.0,
                            base=0, pattern=[[1, 128]], channel_multiplier=-1)
                pending.append((t, ex, lo))
                if len(pending) > 3:   # 3-tile lag so PV never waits on exp
                    emit_pv(*pending.pop(0))
                inject()
            while pending:
                emit_pv(*pending.pop(0))

            # normalization: evacuate PSUM fast (recip + unnormalized copy),
            # then broadcast the reciprocal and scale yT in place — the PE
            # and the acc slots never wait on the broadcast
            denA = smpool.tile([1, 512], F32, tag="denA", name=f"denA{j}_{p}")
            denB = smpool.tile([1, 512], F32, tag="denB", name=f"denB{j}_{p}")
            recA = smpool.tile([1, 512], F32, tag="rec", name=f"recA{j}_{p}")
            recB = smpool.tile([1, 512], F32, tag="rec", name=f"recB{j}_{p}")
            bcA = smpool.tile([128, 512], F32, tag="bcA", name=f"bcA{j}_{p}")
            bcB = smpool.tile([128, 512], F32, tag="bcB", name=f"bcB{j}_{p}")
            last = (j == 3 and p == 3)
            for acc, den, rec, bc, hi in ((accA, denA, recA, bcA, 0),
                                          (accB, denB, recB, bcB, 1)):
                ys = yT[p][64 * hi:64 * hi + 64, q0:q0 + 512]
                nc.vector.tensor_copy(den[:], acc[64:65, :])
                nc.vector.reciprocal_approx_fast(rec[:], den[:])
                if not last:
                    nc.vector.tensor_copy(ys, acc[0:64, :])
                nc.gpsimd.partition_broadcast(bc[:], rec[0:1, :], channels=128)
            # NB: the multiplies must stay on DVE — gpsimd tensor_tensor needs
            # the standard ucode library while partition_broadcast needs attn,
            # and each library swap stalls gpsimd ~7us
            for acc, bc, hi in ((accA, bcA, 0), (accB, bcB, 1)):
                ys = yT[p][64 * hi:64 * hi + 64, q0:q0 + 512]
                if last:
                    # final pair feeds the tail projection: fuse copy+mult
                    # (PSUM operand) to shorten the chain before proj(3)
                    nc.vector.tensor_tensor(ys, acc[0:64, :],
                                            bc[64 * hi:64 * hi + 64, :],
                                            ALU.mult)
                else:
                    nc.vector.tensor_tensor(ys, ys,
                                            bc[64 * hi:64 * hi + 64, :],
                                            ALU.mult)

        # ---- main schedule ---------------------------------------------
        # Segment seg's attention stream absorbs, at sub-tile granularity:
        # this segment's remaining QK projections, the NEXT segment's full
        # QKV (so no PE-only stretches remain between segments), and the
        # PREVIOUS chunk's output projection.
        xqs = [xq0]
        for seg in range(1, 4):
            xqs.append(xpool.tile([128, DC, 512], BF16, name=f"xq{seg}",
                                  tag="xq"))

        for seg in range(4):
            if seg < 3:
                s1 = 512 * (seg + 1)
                nc.sync.dma_start(
                    xqs[seg + 1][:],
                    xT_d[:, s1:s1 + 512].rearrange("(c p) s -> p c s", p=128))
            if seg == 0:   # W_proj isn't read until proj(0) during att(1)
                nc.sync.dma_start(wp_sb[:],
                                  wp_d.rearrange("(c p) f -> p c f", p=128))
            xq = xqs[seg]
            if seg == 0:   # nothing earlier to hide these under
                for u in qk_units(0, 0, xq):
                    u()
                for u in v_units(0, xq):
                    u()
            queues = [[] for _ in range(NPAIR)]
            for pp in (1, 2, 3):
                queues[pp - 1] += qk_units(seg, pp, xq)
            pu = proj_units(seg - 1) if seg >= 1 else []
            nxt = []
            if seg < 3:
                nxt += v_units(seg + 1, xqs[seg + 1])
                nxt += qk_units(seg + 1, 0, xqs[seg + 1])
            queues[1] += pu[0:4] + nxt[0:2]
            queues[2] += pu[4:8] + nxt[2:5]
            queues[3] += nxt[5:8]
            if seg == 3:   # chunk-3 projection partials follow each pair
                for p in range(3):
                    queues[p + 1] += proj3_units(p)
            for p in range(NPAIR):
                q = queues[p]

                def inject(q=q):
                    if q:
                        q.pop(0)()
                att_pair(seg, p, inject)
                while q:   # flush any leftovers at pair end
                    q.pop(0)()
        for u in proj3_units(3):
            u()

    nc.compile()
    return nc


def _get_program():
    if "nc" not in _CACHE:
        _CACHE["nc"] = _build_program()
    return _CACHE["nc"]


def kernel(x, W_attn, b_attn, W_proj, b_proj, _trace=False, _trace_cores=None):
    x = np.asarray(x, np.float32)
    W_attn = np.asarray(W_attn, np.float32)
    b_attn = np.asarray(b_attn, np.float32)
    W_proj = np.asarray(W_proj, np.float32)
    b_proj = np.asarray(b_proj, np.float32)

    nc = _get_program()

    bf16 = ml_dtypes.bfloat16
    x16 = x.astype(bf16)
    Wa16 = W_attn.astype(bf16)
    Wp16 = W_proj.astype(bf16)

    in_maps = []
    for c in range(NCORES):
        b, g = divmod(c, 2)
        gc = slice(FPC * g, FPC * g + FPC)
        in_maps.append({
            "xT": np.ascontiguousarray(x16[b].T),
            "wq": np.ascontiguousarray(Wa16[:, 0 * D:1 * D][:, gc]),
            "wk": np.ascontiguousarray(Wa16[:, 1 * D:2 * D][:, gc]),
            "wv": np.ascontiguousarray(Wa16[:, 2 * D:3 * D][:, gc]),
            "wp": np.ascontiguousarray(Wp16[gc, :]),
            "bq": np.ascontiguousarray(b_attn[0 * D:1 * D][gc]),
            "bk": np.ascontiguousarray(b_attn[1 * D:2 * D][gc]),
        })

    kw = {}
    if _trace:
        kw = dict(trace=True, trace_cores=_trace_cores or [0])
    res = bass_utils.run_bass_kernel_spmd(nc, in_maps, core_ids=list(range(NCORES)),
                                          **kw)

    # host-side reduction: v-bias commutes through softmax -> fold via W_proj
    corr = b_proj + b_attn[2 * D:3 * D] @ W_proj
    out = np.empty((B, S, D), np.float32)
    for b in range(B):
        out[b] = res.results[2 * b]["out"] + res.results[2 * b + 1]["out"] + corr

    if _trace:
        kernel._last_results = res
    return out
